# revision 12
# baseline (speedup 1.0000x reference)
"""ASAP-GNN classifier on trn2 via Bass/Tile.

Architecture (v2): single NeuronCore, device-resident features between
launches. Three compiled programs (NEFFs), built/compiled at import time in
background threads:

  L0  : layer-0 GraphConv + ASAPool attention + LEConv fitness over the
        irregular input graph (slot-table gathers, For_i loops over 157
        row-blocks of 128 nodes).
  L12 : same pipeline for layers 1 and 2 over the fixed-degree kNN graphs
        (shared program; layer-2's 5000 nodes padded to layer-1's shape).
  K   : top-half "select" (gather xn[perm]*fv -> next x + transposed copy +
        running global max) fused with the dense kNN distance scan
        (max8/max_index, two rounds -> 16 neighbor candidates).

Host does only: slot-table construction, top-k via argpartition on the
fitness logits, kNN candidate validation, and the final 1x1536 MLP. Per
layer one launch round-trip for fitness -> perm and one for select+kNN:
6 launches total, ~KBs of traffic each after the initial ~17MB upload.
"""

import math
import threading
import time
import numpy as np

N0 = 20000
IN_CH = 64
HID = 512
OUT = 10
L = 3
RATIO = 0.5

_f32 = np.float32

# ---- geometry constants (hardcoded; program shapes) ----
NB0 = 157                   # layer-0 row blocks
R0 = NB0 * 128              # 20096
X0_ROWS = R0 + 128          # feat_x0 rows (sentinel row = R0, zeros)
D0C_DEFAULT = 17            # layer-0 max in-degree (rebuilt if actual differs)

NB1 = 79                    # layer-1/2 row blocks
R1 = NB1 * 128              # 10112
D1C = 8                     # conv slots for kNN layers (k<=8)
D1P = 9                     # pool slots (self + 8)

XN_ROWS = 20352             # unified xn/x buffer rows (>= R0 + sentinel)
XT_COLS = 10240             # x1T columns (>= R1)
NCH = XT_COLS // 512        # kNN candidate chunks (20)


# ----------------------------------------------------------------------------
# bass plumbing
# ----------------------------------------------------------------------------

_BASS = {}


def _get_bass():
    if not _BASS:
        import concourse.bass as bass
        import concourse.bacc as bacc
        import concourse.mybir as mybir
        from concourse.tile import TileContext
        from concourse.masks import make_identity
        from concourse.bass import ds
        from concourse import bass2jax
        import jax
        import jax.numpy as jnp
        bass2jax.install_neuronx_cc_hook()
        _BASS.update(bass=bass, bacc=bacc, mybir=mybir, TileContext=TileContext,
                     make_identity=make_identity, ds=ds, bass2jax=bass2jax,
                     jax=jax, jnp=jnp)
    return _BASS


class _Launcher:
    """Compiled 1-core bass program; inputs/outputs stay jax device arrays."""

    def __init__(self, nc):
        B = _get_bass()
        jax, jnp, mybir = B["jax"], B["jnp"], B["mybir"]
        bass2jax = B["bass2jax"]
        partition_name = (nc.partition_id_tensor.name
                          if nc.partition_id_tensor else None)
        in_names, in_avals, out_names, out_avals = [], [], [], []
        for alloc in nc.m.functions[0].allocations:
            if not isinstance(alloc, mybir.MemoryLocationSet):
                continue
            name = alloc.memorylocations[0].name
            if alloc.kind == "ExternalInput":
                if name != partition_name:
                    in_names.append(name)
                    in_avals.append(jax.ShapeDtypeStruct(
                        tuple(alloc.tensor_shape), mybir.dt.np(alloc.dtype)))
            elif alloc.kind == "ExternalOutput":
                out_names.append(name)
                out_avals.append(jax.core.ShapedArray(
                    tuple(alloc.tensor_shape), mybir.dt.np(alloc.dtype)))
        self.in_names = in_names
        self.in_avals = in_avals
        self.out_names = out_names
        self.out_avals = out_avals
        n_params = len(in_names)
        all_names = in_names + out_names + (
            [partition_name] if partition_name else [])
        donate = tuple(range(n_params, n_params + len(out_names)))

        def _body(*args):
            operands = list(args)
            if partition_name is not None:
                operands.append(bass2jax.partition_id_tensor())
            outs = bass2jax._bass_exec_p.bind(
                *operands, out_avals=tuple(out_avals),
                in_names=tuple(all_names), out_names=tuple(out_names),
                lowering_input_output_aliases=(),
                sim_require_finite=True, sim_require_nnan=True, nc=nc)
            return tuple(outs)

        self._jit = jax.jit(_body, donate_argnums=donate, keep_unused=True)
        self._compiled = None

    def warm(self):
        """AOT-compile the executable (no execution)."""
        B = _get_bass()
        jax = B["jax"]
        out_structs = [jax.ShapeDtypeStruct(av.shape, av.dtype)
                       for av in self.out_avals]
        self._compiled = self._jit.lower(*self.in_avals,
                                         *out_structs).compile()
        return self

    def __call__(self, in_map):
        B = _get_bass()
        jnp = B["jnp"]
        args = [in_map[nm] for nm in self.in_names]
        zeros = [jnp.zeros(av.shape, av.dtype) for av in self.out_avals]
        fn = self._compiled if self._compiled is not None else self._jit
        outs = fn(*args, *zeros)
        return dict(zip(self.out_names, outs))


# ----------------------------------------------------------------------------
# host helpers
# ----------------------------------------------------------------------------

def _idx_to_i16_tile(idx_list):
    """Compact dma_gather idx tile [16, S]: element m -> partition m%16,
    col m//16. Replicated across the 8 Q7 groups on device."""
    n = len(idx_list)
    S = (n + 15) // 16
    a = np.full((S, 16), -1, np.int16)
    a.reshape(-1)[:n] = idx_list.astype(np.int16)
    return np.ascontiguousarray(a.T)


def _slot_table(src, dst, nrows, D, sentinel):
    """[nrows, D] slot table: row i lists srcs of i's in-edges, sentinel pad."""
    deg = np.bincount(dst, minlength=nrows).astype(np.int64)
    order = np.argsort(dst, kind="stable")
    ss = src[order]
    dsrt = dst[order]
    starts = np.zeros(nrows + 1, np.int64)
    np.cumsum(deg, out=starts[1:])
    slot = np.arange(len(dsrt)) - starts[dsrt]
    tbl = np.full((nrows, D), sentinel, np.int64)
    tbl[dsrt, slot] = ss
    return tbl, deg


def _slotmajor_list(tbl):
    """[rows, D] -> block-slot-major gather list (per 128-block, slot-major)."""
    rows, D = tbl.shape
    nb = rows // 128
    return np.ascontiguousarray(
        tbl.reshape(nb, 128, D).transpose(0, 2, 1)).reshape(-1)


def _rep128(v):
    return np.full((128, 1), v, _f32)


# ----------------------------------------------------------------------------
# program builders
# ----------------------------------------------------------------------------

def _tree_sum(nc, g, n, view):
    """In-place binary-tree reduce over slot axis: view(g, lo, cnt) -> AP.
    Result lands in slot 0. Returns nothing."""
    w = n
    while w > 1:
        h = w // 2
        nc.vector.tensor_add(view(0, h), view(0, h), view(h, h))
        if w % 2:
            nc.vector.tensor_add(view(0, 1), view(0, 1), view(w - 1, 1))
        w = h


def _tree_max(nc, out_t, g, n, gview, oview):
    """Max over n slots of g into out_t (slot tile of n//2 width)."""
    h = n // 2
    nc.vector.tensor_max(oview(0, h), gview(0, h), gview(h, h))
    if n % 2:
        nc.vector.tensor_max(oview(0, 1), oview(0, 1), gview(n - 1, 1))
    w = h
    while w > 1:
        h2 = w // 2
        nc.vector.tensor_max(oview(0, h2), oview(0, h2), oview(h2, h2))
        if w % 2:
            nc.vector.tensor_max(oview(0, 1), oview(0, 1), oview(w - 1, 1))
        w = h2


def _build_L0(D0C):
    """Layer-0: conv + pool + fitness over irregular graph."""
    B = _get_bass()
    bacc, mybir, TileContext = B["bacc"], B["mybir"], B["TileContext"]
    ds, make_identity = B["ds"], B["make_identity"]
    dt = mybir.dt
    D0P = D0C + 1
    SC = NB0 * ((128 * D0C) // 16)       # conv idx tile cols
    SP = NB0 * ((128 * D0P) // 16)       # pool idx tile cols
    F = IN_CH
    HROWS = R0 + 128                     # feat_h rows, sentinel = R0

    nc = bacc.Bacc("TRN2", target_bir_lowering=False)
    x_d = nc.dram_tensor("x", [X0_ROWS, F], dt.float32, kind="ExternalInput")
    cidx_d = nc.dram_tensor("cidx", [16, SC], dt.int16, kind="ExternalInput")
    pidx_d = nc.dram_tensor("pidx", [16, SP], dt.int16, kind="ExternalInput")
    invdeg_d = nc.dram_tensor("invdeg", [R0, 1], dt.float32, kind="ExternalInput")
    cnt_d = nc.dram_tensor("cnt", [R0, 1], dt.float32, kind="ExternalInput")
    wxm_d = nc.dram_tensor("wxm", [128, HID], dt.float32, kind="ExternalInput")
    br_d = nc.dram_tensor("br", [1, HID], dt.float32, kind="ExternalInput")
    wq_d = nc.dram_tensor("wq", [1, HID], dt.float32, kind="ExternalInput")
    aw2_d = nc.dram_tensor("aw2", [1, HID], dt.float32, kind="ExternalInput")
    l1w_d = nc.dram_tensor("l1w", [1, HID], dt.float32, kind="ExternalInput")
    l2w_d = nc.dram_tensor("l2w", [1, HID], dt.float32, kind="ExternalInput")
    l3w_d = nc.dram_tensor("l3w", [1, HID], dt.float32, kind="ExternalInput")
    qb_d = nc.dram_tensor("qb", [128, 1], dt.float32, kind="ExternalInput")
    l1b_d = nc.dram_tensor("l1b", [128, 1], dt.float32, kind="ExternalInput")
    l3b_d = nc.dram_tensor("l3b", [128, 1], dt.float32, kind="ExternalInput")

    h_d = nc.dram_tensor("fh", [HROWS, 576], dt.float32, kind="Internal")
    a_d = nc.dram_tensor("fa", [HROWS, 64], dt.float32, kind="Internal")
    zb_d = nc.dram_tensor("zb", [R0, 1], dt.float32, kind="Internal")
    xn_d = nc.dram_tensor("xn", [XN_ROWS, HID], dt.float32,
                          kind="ExternalOutput")
    z_d = nc.dram_tensor("z", [R0, 1], dt.float32, kind="ExternalOutput")

    with TileContext(nc) as tc:
        with (
            tc.tile_pool(name="const", bufs=1) as cpool,
            tc.tile_pool(name="ps", bufs=2, space="PSUM") as pspool,
        ):
            ident = cpool.tile([128, 128], dt.float32)
            make_identity(nc, ident[:])
            wxm_sb = cpool.tile([128, HID], dt.float32)
            nc.sync.dma_start(wxm_sb[:], wxm_d[:, :])
            br_sb = cpool.tile([128, HID], dt.float32)
            nc.sync.dma_start(br_sb[:], br_d[0:1, :].to_broadcast([128, HID]))
            wq_sb = cpool.tile([128, HID], dt.float32)
            nc.sync.dma_start(wq_sb[:], wq_d[0:1, :].to_broadcast([128, HID]))
            aw2_sb = cpool.tile([128, HID], dt.float32)
            nc.sync.dma_start(aw2_sb[:], aw2_d[0:1, :].to_broadcast([128, HID]))
            l1w_sb = cpool.tile([128, HID], dt.float32)
            nc.sync.dma_start(l1w_sb[:], l1w_d[0:1, :].to_broadcast([128, HID]))
            l2w_sb = cpool.tile([128, HID], dt.float32)
            nc.sync.dma_start(l2w_sb[:], l2w_d[0:1, :].to_broadcast([128, HID]))
            l3w_sb = cpool.tile([128, HID], dt.float32)
            nc.sync.dma_start(l3w_sb[:], l3w_d[0:1, :].to_broadcast([128, HID]))
            qb_sb = cpool.tile([128, 1], dt.float32)
            nc.sync.dma_start(qb_sb[:], qb_d[:, :])
            l1b_sb = cpool.tile([128, 1], dt.float32)
            nc.sync.dma_start(l1b_sb[:], l1b_d[:, :])
            l3b_sb = cpool.tile([128, 1], dt.float32)
            nc.sync.dma_start(l3b_sb[:], l3b_d[:, :])
            pidx_sb = cpool.tile([128, SP], dt.int16)
            for _g in range(8):
                nc.sync.dma_start(pidx_sb[_g * 16:(_g + 1) * 16, :],
                                  pidx_d[:, :])
            # sentinel rows: feat_h[R0] = zeros except js col = -1e30;
            # feat_a[R0] = 0
            srow = cpool.tile([1, 576], dt.float32)
            nc.vector.memset(srow[:], 0.0)
            nc.vector.memset(srow[:, 512:513], -1e30)
            nc.sync.dma_start(h_d[R0:R0 + 1, :], srow[:])
            nc.sync.dma_start(a_d[R0:R0 + 1, :], srow[:, 0:64])

            # ---- phase A: conv ----
            SCB = (128 * D0C) // 16
            with tc.tile_pool(name="conv", bufs=2) as wp:
                cidx_sb = wp.tile([128, SC], dt.int16, tag="cidx", bufs=1)
                for _g in range(8):
                    nc.sync.dma_start(cidx_sb[_g * 16:(_g + 1) * 16, :],
                                      cidx_d[:, :])

                def conv_body(i):
                    g = wp.tile([128, D0C, F], dt.float32, tag="g")
                    nc.gpsimd.dma_gather(
                        out_ap=g[:], in_ap=x_d[:, :],
                        idxs_ap=cidx_sb[:, ds(i * SCB, SCB)],
                        num_idxs=128 * D0C, num_idxs_reg=128 * D0C,
                        elem_size=F, single_packet=False)
                    _tree_sum(nc, g, D0C,
                              lambda lo, cnt: g[:, lo:lo + cnt, :])
                    iv = wp.tile([128, 1], dt.float32, tag="iv")
                    nc.sync.dma_start(iv[:], invdeg_d[ds(i * 128, 128), :])
                    xm = wp.tile([128, 128], dt.float32, tag="xm")
                    nc.sync.dma_start(xm[:, 0:F], x_d[ds(i * 128, 128), :])
                    nc.vector.tensor_scalar_mul(xm[:, F:2 * F], g[:, 0, :],
                                                iv[:])
                    tp = pspool.tile([128, 128], dt.float32, tag="tp")
                    nc.tensor.transpose(tp[:], xm[:], ident[:])
                    lhsT = wp.tile([128, 128], dt.float32, tag="lhsT")
                    nc.vector.tensor_copy(lhsT[:], tp[:])
                    hps = pspool.tile([128, HID], dt.float32, tag="hps")
                    nc.tensor.matmul(hps[:], lhsT[:], wxm_sb[:],
                                     start=True, stop=True)
                    hsb = wp.tile([128, 576], dt.float32, tag="hsb")
                    nc.vector.tensor_add(
                        hsb[:, 0:HID], hps[:],
                        br_sb[:])
                    nc.vector.tensor_scalar_max(hsb[:, 0:HID], hsb[:, 0:HID],
                                                0.0)
                    tmp = wp.tile([128, HID], dt.float32, tag="tmp")
                    nc.vector.tensor_mul(tmp[:], hsb[:, 0:HID],
                                         aw2_sb[:])
                    nc.vector.tensor_reduce(hsb[:, 512:513], tmp[:],
                                            axis=mybir.AxisListType.X,
                                            op=mybir.AluOpType.add)
                    nc.sync.dma_start(h_d[ds(i * 128, 128), 0:513],
                                      hsb[:, 0:513])
                tc.For_i_unrolled(0, NB0, 1, conv_body, max_unroll=2)

            # ---- phase B: pool ----
            SPB = (128 * D0P) // 16
            with tc.tile_pool(name="pool", bufs=2) as wp:
                def pool_body(i):
                    g = wp.tile([128, D0P, 576], dt.float32, tag="g")
                    nc.gpsimd.dma_gather(
                        out_ap=g[:], in_ap=h_d[:, :],
                        idxs_ap=pidx_sb[:, ds(i * SPB, SPB)],
                        num_idxs=128 * D0P, num_idxs_reg=128 * D0P,
                        elem_size=576, single_packet=False)
                    xq = wp.tile([128, D0P // 2, HID], dt.float32, tag="xq")
                    _tree_max(nc, xq, g, D0P,
                              lambda lo, cnt: g[:, lo:lo + cnt, 0:HID],
                              lambda lo, cnt: xq[:, lo:lo + cnt, :])
                    tmp = wp.tile([128, HID], dt.float32, tag="tmp")
                    nc.vector.tensor_mul(tmp[:], xq[:, 0, :],
                                         wq_sb[:])
                    qs = wp.tile([128, 1], dt.float32, tag="qs")
                    nc.vector.tensor_reduce(qs[:], tmp[:],
                                            axis=mybir.AxisListType.X,
                                            op=mybir.AluOpType.add)
                    nc.vector.tensor_add(qs[:], qs[:], qb_sb[:])
                    # score = leaky_relu(qs + js)
                    sc = wp.tile([128, D0P], dt.float32, tag="sc")
                    jsv = g[:, :, 512:513].squeeze(2)
                    nc.vector.tensor_scalar_add(sc[:], jsv, qs[:])
                    sc2 = wp.tile([128, D0P], dt.float32, tag="sc2")
                    nc.vector.tensor_scalar_mul(sc2[:], sc[:], 0.2)
                    nc.vector.tensor_max(sc[:], sc[:], sc2[:])
                    m = wp.tile([128, 1], dt.float32, tag="m")
                    nc.vector.tensor_reduce(m[:], sc[:],
                                            axis=mybir.AxisListType.X,
                                            op=mybir.AluOpType.max)
                    nc.vector.tensor_scalar(sc[:], sc[:], m[:], None,
                                            op0=mybir.AluOpType.subtract)
                    nc.scalar.activation(sc[:], sc[:],
                                         mybir.ActivationFunctionType.Exp)
                    ssum = wp.tile([128, 1], dt.float32, tag="ssum")
                    nc.vector.tensor_reduce(ssum[:], sc[:],
                                            axis=mybir.AxisListType.X,
                                            op=mybir.AluOpType.add)
                    rec = wp.tile([128, 1], dt.float32, tag="rec")
                    nc.vector.reciprocal(rec[:], ssum[:])
                    nc.vector.tensor_scalar_mul(sc[:], sc[:], rec[:])
                    # xn = sum_s att_s * h_s  (scale slots in place, tree add)
                    gh = g[:, :, 0:HID]
                    nc.vector.tensor_mul(
                        gh, gh, sc[:].unsqueeze(2).to_broadcast(
                            [128, D0P, HID]))
                    _tree_sum(nc, g, D0P,
                              lambda lo, cnt: g[:, lo:lo + cnt, 0:HID])
                    xn = g[:, 0, 0:HID]
                    nc.sync.dma_start(xn_d[ds(i * 128, 128), :], xn)
                    # fitness scalars
                    nc.vector.tensor_mul(tmp[:], xn,
                                         l1w_sb[:])
                    av = wp.tile([128, 1], dt.float32, tag="av")
                    nc.vector.tensor_reduce(av[:], tmp[:],
                                            axis=mybir.AxisListType.X,
                                            op=mybir.AluOpType.add)
                    nc.sync.dma_start(a_d[ds(i * 128, 128), 0:1], av[:])
                    nc.vector.tensor_mul(tmp[:], xn,
                                         l2w_sb[:])
                    bv = wp.tile([128, 1], dt.float32, tag="bv")
                    nc.vector.tensor_reduce(bv[:], tmp[:],
                                            axis=mybir.AxisListType.X,
                                            op=mybir.AluOpType.add)
                    nc.vector.tensor_mul(tmp[:], xn,
                                         l3w_sb[:])
                    cv = wp.tile([128, 1], dt.float32, tag="cv")
                    nc.vector.tensor_reduce(cv[:], tmp[:],
                                            axis=mybir.AxisListType.X,
                                            op=mybir.AluOpType.add)
                    ct = wp.tile([128, 1], dt.float32, tag="ct")
                    nc.sync.dma_start(ct[:], cnt_d[ds(i * 128, 128), :])
                    # zb = c + l3b - cnt*b + cnt*l1b
                    zb = wp.tile([128, 1], dt.float32, tag="zb")
                    nc.vector.tensor_mul(zb[:], ct[:], bv[:])
                    nc.vector.tensor_sub(zb[:], cv[:], zb[:])
                    nc.vector.tensor_add(zb[:], zb[:], l3b_sb[:])
                    lb1 = wp.tile([128, 1], dt.float32, tag="lb1")
                    nc.vector.tensor_mul(lb1[:], ct[:], l1b_sb[:])
                    nc.vector.tensor_add(zb[:], zb[:], lb1[:])
                    nc.sync.dma_start(zb_d[ds(i * 128, 128), :], zb[:])
                tc.For_i_unrolled(0, NB0, 1, pool_body, max_unroll=2)

            # ---- phase C: fitness gather ----
            with tc.tile_pool(name="fit", bufs=2) as wp:
                def fit_body(i):
                    ga = wp.tile([128, D0P, 64], dt.float32, tag="ga")
                    nc.gpsimd.dma_gather(
                        out_ap=ga[:], in_ap=a_d[:, :],
                        idxs_ap=pidx_sb[:, ds(i * SPB, SPB)],
                        num_idxs=128 * D0P, num_idxs_reg=128 * D0P,
                        elem_size=64, single_packet=False)
                    zs = wp.tile([128, 1], dt.float32, tag="zs")
                    nc.vector.tensor_reduce(zs[:], ga[:, :, 0:1].squeeze(2),
                                            axis=mybir.AxisListType.X,
                                            op=mybir.AluOpType.add)
                    zb = wp.tile([128, 1], dt.float32, tag="zb2")
                    nc.sync.dma_start(zb[:], zb_d[ds(i * 128, 128), :])
                    nc.vector.tensor_add(zs[:], zs[:], zb[:])
                    nc.sync.dma_start(z_d[ds(i * 128, 128), :], zs[:])
                tc.For_i_unrolled(0, NB0, 1, fit_body, max_unroll=4)
    nc.compile()
    return nc


def _build_L12():
    """Layers 1/2: conv + pool + fitness over fixed-degree kNN graph."""
    B = _get_bass()
    bacc, mybir, TileContext = B["bacc"], B["mybir"], B["TileContext"]
    ds, make_identity = B["ds"], B["make_identity"]
    dt = mybir.dt
    F = HID
    SC = NB1 * ((128 * D1C) // 16)
    SP = NB1 * ((128 * D1P) // 16)
    HROWS = R1 + 128                    # sentinel = R1

    nc = bacc.Bacc("TRN2", target_bir_lowering=False)
    x_d = nc.dram_tensor("x", [XN_ROWS, F], dt.float32, kind="ExternalInput")
    xT_d = nc.dram_tensor("xT", [F, XT_COLS], dt.float32, kind="ExternalInput")
    cidx_d = nc.dram_tensor("cidx", [16, SC], dt.int16, kind="ExternalInput")
    pidx_d = nc.dram_tensor("pidx", [16, SP], dt.int16, kind="ExternalInput")
    invdeg_d = nc.dram_tensor("invdeg", [128, 1], dt.float32,
                              kind="ExternalInput")
    cnt_d = nc.dram_tensor("cnt", [128, 1], dt.float32, kind="ExternalInput")
    wr_d = nc.dram_tensor("wr", [128, 4, HID], dt.float32,
                          kind="ExternalInput")
    wl_d = nc.dram_tensor("wl", [128, 4, HID], dt.float32,
                          kind="ExternalInput")
    br_d = nc.dram_tensor("br", [1, HID], dt.float32, kind="ExternalInput")
    wq_d = nc.dram_tensor("wq", [1, HID], dt.float32, kind="ExternalInput")
    aw2_d = nc.dram_tensor("aw2", [1, HID], dt.float32, kind="ExternalInput")
    l1w_d = nc.dram_tensor("l1w", [1, HID], dt.float32, kind="ExternalInput")
    l2w_d = nc.dram_tensor("l2w", [1, HID], dt.float32, kind="ExternalInput")
    l3w_d = nc.dram_tensor("l3w", [1, HID], dt.float32, kind="ExternalInput")
    qb_d = nc.dram_tensor("qb", [128, 1], dt.float32, kind="ExternalInput")
    l1b_d = nc.dram_tensor("l1b", [128, 1], dt.float32, kind="ExternalInput")
    l3b_d = nc.dram_tensor("l3b", [128, 1], dt.float32, kind="ExternalInput")

    h_d = nc.dram_tensor("fh", [HROWS, 576], dt.float32, kind="Internal")
    a_d = nc.dram_tensor("fa", [HROWS, 64], dt.float32, kind="Internal")
    zb_d = nc.dram_tensor("zb", [R1, 1], dt.float32, kind="Internal")
    xn_d = nc.dram_tensor("xn", [XN_ROWS, HID], dt.float32,
                          kind="ExternalOutput")
    z_d = nc.dram_tensor("z", [R1, 1], dt.float32, kind="ExternalOutput")

    with TileContext(nc) as tc:
        with (
            tc.tile_pool(name="const", bufs=1) as cpool,
            tc.tile_pool(name="ps", bufs=2, space="PSUM") as pspool,
        ):
            ident = cpool.tile([128, 128], dt.float32)
            make_identity(nc, ident[:])
            wr_sb = cpool.tile([128, 4, HID], dt.float32)
            nc.sync.dma_start(wr_sb[:], wr_d[:, :, :])
            wl_sb = cpool.tile([128, 4, HID], dt.float32)
            nc.sync.dma_start(wl_sb[:], wl_d[:, :, :])
            br_sb = cpool.tile([128, HID], dt.float32)
            nc.sync.dma_start(br_sb[:], br_d[0:1, :].to_broadcast([128, HID]))
            wq_sb = cpool.tile([128, HID], dt.float32)
            nc.sync.dma_start(wq_sb[:], wq_d[0:1, :].to_broadcast([128, HID]))
            aw2_sb = cpool.tile([128, HID], dt.float32)
            nc.sync.dma_start(aw2_sb[:], aw2_d[0:1, :].to_broadcast([128, HID]))
            l1w_sb = cpool.tile([128, HID], dt.float32)
            nc.sync.dma_start(l1w_sb[:], l1w_d[0:1, :].to_broadcast([128, HID]))
            l2w_sb = cpool.tile([128, HID], dt.float32)
            nc.sync.dma_start(l2w_sb[:], l2w_d[0:1, :].to_broadcast([128, HID]))
            l3w_sb = cpool.tile([128, HID], dt.float32)
            nc.sync.dma_start(l3w_sb[:], l3w_d[0:1, :].to_broadcast([128, HID]))
            qb_sb = cpool.tile([128, 1], dt.float32)
            nc.sync.dma_start(qb_sb[:], qb_d[:, :])
            l1b_sb = cpool.tile([128, 1], dt.float32)
            nc.sync.dma_start(l1b_sb[:], l1b_d[:, :])
            l3b_sb = cpool.tile([128, 1], dt.float32)
            nc.sync.dma_start(l3b_sb[:], l3b_d[:, :])
            iv_sb = cpool.tile([128, 1], dt.float32)
            nc.sync.dma_start(iv_sb[:], invdeg_d[:, :])
            ct_sb = cpool.tile([128, 1], dt.float32)
            nc.sync.dma_start(ct_sb[:], cnt_d[:, :])
            cidx_sb = cpool.tile([128, SC], dt.int16)
            for _g in range(8):
                nc.sync.dma_start(cidx_sb[_g * 16:(_g + 1) * 16, :],
                                  cidx_d[:, :])
            pidx_sb = cpool.tile([128, SP], dt.int16)
            for _g in range(8):
                nc.sync.dma_start(pidx_sb[_g * 16:(_g + 1) * 16, :],
                                  pidx_d[:, :])
            srow = cpool.tile([1, 576], dt.float32)
            nc.vector.memset(srow[:], 0.0)
            nc.vector.memset(srow[:, 512:513], -1e30)
            nc.sync.dma_start(h_d[R1:R1 + 1, :], srow[:])
            nc.sync.dma_start(a_d[R1:R1 + 1, :], srow[:, 0:64])

            SCB = (128 * D1C) // 16
            SPB = (128 * D1P) // 16
            with tc.tile_pool(name="conv", bufs=2) as wp:
                def conv_body(i):
                    g = wp.tile([128, D1C, F], dt.float32, tag="g")
                    nc.gpsimd.dma_gather(
                        out_ap=g[:], in_ap=x_d[:, :],
                        idxs_ap=cidx_sb[:, ds(i * SCB, SCB)],
                        num_idxs=128 * D1C, num_idxs_reg=128 * D1C,
                        elem_size=F, single_packet=False)
                    _tree_sum(nc, g, D1C,
                              lambda lo, cnt: g[:, lo:lo + cnt, :])
                    mean = wp.tile([128, F], dt.float32, tag="mean")
                    nc.vector.tensor_scalar_mul(mean[:], g[:, 0, :], iv_sb[:])
                    hps = pspool.tile([128, HID], dt.float32, tag="hps")
                    xt = wp.tile([128, 4, 128], dt.float32, tag="xt")
                    nc.sync.dma_start(
                        xt[:], xT_d[:, ds(i * 128, 128)].rearrange(
                            "(c r) m -> r c m", c=4))
                    mt = wp.tile([128, 4, 128], dt.float32, tag="mt")
                    for c in range(4):
                        tp = pspool.tile([128, 128], dt.float32, tag="tp")
                        nc.tensor.transpose(tp[:],
                                            mean[:, c * 128:(c + 1) * 128],
                                            ident[:])
                        nc.vector.tensor_copy(mt[:, c, :], tp[:])
                    for c in range(4):
                        nc.tensor.matmul(hps[:], xt[:, c, :], wl_sb[:, c, :],
                                         start=(c == 0), stop=False)
                    for c in range(4):
                        nc.tensor.matmul(hps[:], mt[:, c, :], wr_sb[:, c, :],
                                         start=False, stop=(c == 3))
                    hsb = wp.tile([128, 576], dt.float32, tag="hsb")
                    nc.vector.tensor_add(
                        hsb[:, 0:HID], hps[:],
                        br_sb[:])
                    nc.vector.tensor_scalar_max(hsb[:, 0:HID], hsb[:, 0:HID],
                                                0.0)
                    tmp = wp.tile([128, HID], dt.float32, tag="tmp")
                    nc.vector.tensor_mul(tmp[:], hsb[:, 0:HID],
                                         aw2_sb[:])
                    nc.vector.tensor_reduce(hsb[:, 512:513], tmp[:],
                                            axis=mybir.AxisListType.X,
                                            op=mybir.AluOpType.add)
                    nc.sync.dma_start(h_d[ds(i * 128, 128), 0:513],
                                      hsb[:, 0:513])
                tc.For_i_unrolled(0, NB1, 1, conv_body, max_unroll=2)

            with tc.tile_pool(name="pool", bufs=2) as wp:
                def pool_body(i):
                    g = wp.tile([128, D1P, 576], dt.float32, tag="g")
                    nc.gpsimd.dma_gather(
                        out_ap=g[:], in_ap=h_d[:, :],
                        idxs_ap=pidx_sb[:, ds(i * SPB, SPB)],
                        num_idxs=128 * D1P, num_idxs_reg=128 * D1P,
                        elem_size=576, single_packet=False)
                    xq = wp.tile([128, D1P // 2, HID], dt.float32, tag="xq")
                    _tree_max(nc, xq, g, D1P,
                              lambda lo, cnt: g[:, lo:lo + cnt, 0:HID],
                              lambda lo, cnt: xq[:, lo:lo + cnt, :])
                    tmp = wp.tile([128, HID], dt.float32, tag="tmp")
                    nc.vector.tensor_mul(tmp[:], xq[:, 0, :],
                                         wq_sb[:])
                    qs = wp.tile([128, 1], dt.float32, tag="qs")
                    nc.vector.tensor_reduce(qs[:], tmp[:],
                                            axis=mybir.AxisListType.X,
                                            op=mybir.AluOpType.add)
                    nc.vector.tensor_add(qs[:], qs[:], qb_sb[:])
                    sc = wp.tile([128, D1P], dt.float32, tag="sc")
                    jsv = g[:, :, 512:513].squeeze(2)
                    nc.vector.tensor_scalar_add(sc[:], jsv, qs[:])
                    sc2 = wp.tile([128, D1P], dt.float32, tag="sc2")
                    nc.vector.tensor_scalar_mul(sc2[:], sc[:], 0.2)
                    nc.vector.tensor_max(sc[:], sc[:], sc2[:])
                    m = wp.tile([128, 1], dt.float32, tag="m")
                    nc.vector.tensor_reduce(m[:], sc[:],
                                            axis=mybir.AxisListType.X,
                                            op=mybir.AluOpType.max)
                    nc.vector.tensor_scalar(sc[:], sc[:], m[:], None,
                                            op0=mybir.AluOpType.subtract)
                    nc.scalar.activation(sc[:], sc[:],
                                         mybir.ActivationFunctionType.Exp)
                    ssum = wp.tile([128, 1], dt.float32, tag="ssum")
                    nc.vector.tensor_reduce(ssum[:], sc[:],
                                            axis=mybir.AxisListType.X,
                                            op=mybir.AluOpType.add)
                    rec = wp.tile([128, 1], dt.float32, tag="rec")
                    nc.vector.reciprocal(rec[:], ssum[:])
                    nc.vector.tensor_scalar_mul(sc[:], sc[:], rec[:])
                    gh = g[:, :, 0:HID]
                    nc.vector.tensor_mul(
                        gh, gh, sc[:].unsqueeze(2).to_broadcast(
                            [128, D1P, HID]))
                    _tree_sum(nc, g, D1P,
                              lambda lo, cnt: g[:, lo:lo + cnt, 0:HID])
                    xn = g[:, 0, 0:HID]
                    nc.sync.dma_start(xn_d[ds(i * 128, 128), :], xn)
                    nc.vector.tensor_mul(tmp[:], xn,
                                         l1w_sb[:])
                    av = wp.tile([128, 1], dt.float32, tag="av")
                    nc.vector.tensor_reduce(av[:], tmp[:],
                                            axis=mybir.AxisListType.X,
                                            op=mybir.AluOpType.add)
                    nc.sync.dma_start(a_d[ds(i * 128, 128), 0:1], av[:])
                    nc.vector.tensor_mul(tmp[:], xn,
                                         l2w_sb[:])
                    bv = wp.tile([128, 1], dt.float32, tag="bv")
                    nc.vector.tensor_reduce(bv[:], tmp[:],
                                            axis=mybir.AxisListType.X,
                                            op=mybir.AluOpType.add)
                    nc.vector.tensor_mul(tmp[:], xn,
                                         l3w_sb[:])
                    cv = wp.tile([128, 1], dt.float32, tag="cv")
                    nc.vector.tensor_reduce(cv[:], tmp[:],
                                            axis=mybir.AxisListType.X,
                                            op=mybir.AluOpType.add)
                    zb = wp.tile([128, 1], dt.float32, tag="zb")
                    nc.vector.tensor_mul(zb[:], ct_sb[:], bv[:])
                    nc.vector.tensor_sub(zb[:], cv[:], zb[:])
                    nc.vector.tensor_add(zb[:], zb[:], l3b_sb[:])
                    lb1 = wp.tile([128, 1], dt.float32, tag="lb1")
                    nc.vector.tensor_mul(lb1[:], ct_sb[:], l1b_sb[:])
                    nc.vector.tensor_add(zb[:], zb[:], lb1[:])
                    nc.sync.dma_start(zb_d[ds(i * 128, 128), :], zb[:])
                tc.For_i_unrolled(0, NB1, 1, pool_body, max_unroll=2)

            with tc.tile_pool(name="fit", bufs=2) as wp:
                def fit_body(i):
                    ga = wp.tile([128, D1P, 64], dt.float32, tag="ga")
                    nc.gpsimd.dma_gather(
                        out_ap=ga[:], in_ap=a_d[:, :],
                        idxs_ap=pidx_sb[:, ds(i * SPB, SPB)],
                        num_idxs=128 * D1P, num_idxs_reg=128 * D1P,
                        elem_size=64, single_packet=False)
                    zs = wp.tile([128, 1], dt.float32, tag="zs")
                    nc.vector.tensor_reduce(zs[:], ga[:, :, 0:1].squeeze(2),
                                            axis=mybir.AxisListType.X,
                                            op=mybir.AluOpType.add)
                    zb = wp.tile([128, 1], dt.float32, tag="zb2")
                    nc.sync.dma_start(zb[:], zb_d[ds(i * 128, 128), :])
                    nc.vector.tensor_add(zs[:], zs[:], zb[:])
                    nc.sync.dma_start(z_d[ds(i * 128, 128), :], zs[:])
                tc.For_i_unrolled(0, NB1, 1, fit_body, max_unroll=4)
    nc.compile()
    return nc


def _build_K():
    """Select (gather xn[perm]*fv -> x, xT, running max) + kNN scan."""
    B = _get_bass()
    bacc, mybir, TileContext = B["bacc"], B["mybir"], B["TileContext"]
    ds, make_identity = B["ds"], B["make_identity"]
    dt = mybir.dt
    SS = NB1 * ((128 * 1) // 16)   # select idx cols (1 slot per row)

    nc = bacc.Bacc("TRN2", target_bir_lowering=False)
    xn_d = nc.dram_tensor("xn", [XN_ROWS, HID], dt.float32,
                          kind="ExternalInput")
    sidx_d = nc.dram_tensor("sidx", [16, SS], dt.int16, kind="ExternalInput")
    fv_d = nc.dram_tensor("fv", [R1, 1], dt.float32, kind="ExternalInput")
    msk_d = nc.dram_tensor("msk", [R1, 1], dt.float32, kind="ExternalInput")
    qT_d = nc.dram_tensor("qT", [4, XT_COLS], dt.float32, kind="ExternalInput")
    cand_d = nc.dram_tensor("cand", [4, XT_COLS], dt.float32,
                            kind="ExternalInput")
    x_d = nc.dram_tensor("xo", [XN_ROWS, HID], dt.float32,
                         kind="ExternalOutput")
    xT_d = nc.dram_tensor("xT", [HID, XT_COLS], dt.float32,
                          kind="ExternalOutput")
    xsp_d = nc.dram_tensor("xsp", [128, HID], dt.float32,
                           kind="ExternalOutput")
    knn_d = nc.dram_tensor("knn", [R1, 16], dt.uint16, kind="ExternalOutput")

    with TileContext(nc) as tc:
        with (
            tc.tile_pool(name="const", bufs=1) as cpool,
            tc.tile_pool(name="ps", bufs=2, space="PSUM") as pspool,
        ):
            ident = cpool.tile([128, 128], dt.float32)
            make_identity(nc, ident[:])
            sidx_sb = cpool.tile([128, SS], dt.int16)
            for _g in range(8):
                nc.sync.dma_start(sidx_sb[_g * 16:(_g + 1) * 16, :],
                                  sidx_d[:, :])
            cand_sb = cpool.tile([4, XT_COLS], dt.float32)
            nc.sync.dma_start(cand_sb[:], cand_d[:, :])
            runmax = cpool.tile([128, HID], dt.float32)
            nc.vector.memset(runmax[:], -1e30)

            SSB = 8   # (128*1)//16
            with tc.tile_pool(name="sel", bufs=2) as wp:
                def sel_body(i):
                    g = wp.tile([128, 1, HID], dt.float32, tag="g")
                    nc.gpsimd.dma_gather(
                        out_ap=g[:], in_ap=xn_d[:, :],
                        idxs_ap=sidx_sb[:, ds(i * SSB, SSB)],
                        num_idxs=128, num_idxs_reg=128,
                        elem_size=HID, single_packet=False)
                    fv = wp.tile([128, 1], dt.float32, tag="fv")
                    nc.sync.dma_start(fv[:], fv_d[ds(i * 128, 128), :])
                    xs = wp.tile([128, HID], dt.float32, tag="xs")
                    nc.vector.tensor_scalar_mul(xs[:], g[:, 0, :], fv[:])
                    nc.sync.dma_start(x_d[ds(i * 128, 128), :], xs[:])
                    mk = wp.tile([128, 1], dt.float32, tag="mk")
                    nc.sync.dma_start(mk[:], msk_d[ds(i * 128, 128), :])
                    xm = wp.tile([128, HID], dt.float32, tag="xm2")
                    nc.vector.tensor_scalar_add(xm[:], xs[:], mk[:])
                    nc.vector.tensor_max(runmax[:], runmax[:], xm[:])
                    for c in range(4):
                        tp = pspool.tile([128, 128], dt.float32, tag="tp")
                        nc.tensor.transpose(tp[:],
                                            xs[:, c * 128:(c + 1) * 128],
                                            ident[:])
                        tt = wp.tile([128, 128], dt.float32, tag="tt")
                        nc.vector.tensor_copy(tt[:], tp[:])
                        nc.sync.dma_start(
                            xT_d[c * 128:(c + 1) * 128, ds(i * 128, 128)],
                            tt[:])
                tc.For_i_unrolled(0, NB1, 1, sel_body, max_unroll=2)
            nc.sync.dma_start(xsp_d[:, :], runmax[:])

            with tc.tile_pool(name="knn", bufs=2) as wp:
                def knn_body(i):
                    qsb = wp.tile([4, 128], dt.float32, tag="q")
                    nc.sync.dma_start(qsb[:], qT_d[:, ds(i * 128, 128)])
                    row = wp.tile([128, XT_COLS], dt.float32, tag="row")
                    for ch in range(NCH):
                        dps = pspool.tile([128, 512], dt.float32, tag="d")
                        nc.tensor.matmul(dps[:], qsb[:],
                                         cand_sb[:, ch * 512:(ch + 1) * 512],
                                         start=True, stop=True)
                        nc.scalar.activation(
                            row[:, ch * 512:(ch + 1) * 512], dps[:],
                            mybir.ActivationFunctionType.Copy)
                    v8 = wp.tile([128, 8], dt.float32, tag="v8")
                    nc.vector.max(out=v8[:], in_=row[:])
                    i16 = wp.tile([128, 16], dt.uint16, tag="i16")
                    i8 = wp.tile([128, 8], dt.uint32, tag="i8")
                    nc.vector.max_index(i8[:], v8[:], row[:])
                    nc.vector.tensor_copy(i16[:, 0:8], i8[:])
                    nc.vector.match_replace(out=row[:], in_to_replace=v8[:],
                                            in_values=row[:], imm_value=-3e30)
                    v8b = wp.tile([128, 8], dt.float32, tag="v8b")
                    nc.vector.max(out=v8b[:], in_=row[:])
                    i8b = wp.tile([128, 8], dt.uint32, tag="i8b")
                    nc.vector.max_index(i8b[:], v8b[:], row[:])
                    nc.vector.tensor_copy(i16[:, 8:16], i8b[:])
                    nc.sync.dma_start(knn_d[ds(i * 128, 128), :], i16[:])
                tc.For_i_unrolled(0, NB1, 1, knn_body, max_unroll=2)
    nc.compile()
    return nc


# ----------------------------------------------------------------------------
# build/compile management (import-time warm-up)
# ----------------------------------------------------------------------------

_RUNNERS = {}
_BUILD_LOCK = threading.Lock()
_BUILD_THREADS = []


def _get_runner(name, builder):
    with _BUILD_LOCK:
        if name in _RUNNERS:
            return _RUNNERS[name]
    r = _Launcher(builder()).warm()
    with _BUILD_LOCK:
        _RUNNERS.setdefault(name, r)
    return _RUNNERS[name]


def _warm():
    try:
        B = _get_bass()
        jnp = B["jnp"]
        ncs = {}
        ncs["L0_%d" % D0C_DEFAULT] = _build_L0(D0C_DEFAULT)
        ncs["L12"] = _build_L12()
        ncs["K"] = _build_K()
        launchers = {name: _Launcher(nc) for name, nc in ncs.items()}
        # compile the programs and the donated-zeros broadcast kernels in
        # parallel (each is a neuronx-cc subprocess)
        shapes = {}
        for l in launchers.values():
            for av in l.out_avals:
                shapes[(av.shape, str(av.dtype))] = av
        ths = [threading.Thread(target=l.warm) for l in launchers.values()]
        ths += [threading.Thread(
            target=lambda a=av: jnp.zeros(a.shape, a.dtype).block_until_ready())
            for av in shapes.values()]
        for t in ths:
            t.start()
        for t in ths:
            t.join()
        with _BUILD_LOCK:
            for name, l in launchers.items():
                _RUNNERS.setdefault(name, l)
    except Exception:  # pragma: no cover - fallback path handles
        import traceback
        traceback.print_exc()


_BUILD_THREADS.append(threading.Thread(target=_warm, daemon=True))
_BUILD_THREADS[-1].start()


# ----------------------------------------------------------------------------
# numpy fallbacks (used only if the device path fails)
# ----------------------------------------------------------------------------

def _np_reference(x, pos, src, dst, W):
    f = _f32
    n = N0
    xs = []
    for i in range(L):
        wr, br, wl = W["wr"][i], W["br"][i], W["wl"][i]
        agg = np.zeros((n, x.shape[1]), f)
        np.add.at(agg, dst, x[src])
        deg = np.bincount(dst, minlength=n).astype(f)
        mean = agg / np.maximum(deg, 1)[:, None]
        h = np.maximum(mean @ wr + br + x @ wl, 0).astype(f)
        sl = np.arange(n)
        s_ = np.concatenate([src, sl])
        d_ = np.concatenate([dst, sl])
        xj = h[s_]
        xq = np.full((n, HID), -np.inf, f)
        np.maximum.at(xq, d_, xj)
        xq = (xq @ W["lw"][i] + W["lb"][i]).astype(f)
        aw, ab = W["aw"][i], W["ab"][i]
        score = (xq[d_] @ aw[:HID] + xj @ aw[HID:] + ab).astype(f)
        score = np.where(score > 0, score, f(0.2) * score).astype(f)
        smax = np.full(n, -np.inf, f)
        np.maximum.at(smax, d_, score)
        ex = np.exp(score - smax[d_])
        ssum = np.zeros(n, f)
        np.add.at(ssum, d_, ex)
        att = (ex / ssum[d_]).astype(f)
        xn = np.zeros((n, HID), f)
        np.add.at(xn, d_, xj * att[:, None])
        a = xn @ W["l1w"][i] + W["l1b"][i]
        b = xn @ W["l2w"][i]
        agg2 = np.zeros(n, f)
        np.add.at(agg2, d_, (a[s_] - b[d_]).astype(f))
        z = (agg2 + xn @ W["l3w"][i] + W["l3b"][i]).astype(f)
        k_keep = int(math.ceil(RATIO * n))
        fit64 = 1.0 / (1.0 + np.exp(-z.astype(np.float64)))
        perm = np.argpartition(-fit64, k_keep - 1)[:k_keep]
        fv = fit64[perm].astype(f)
        x = (xn[perm] * fv[:, None]).astype(f)
        xs.append(x.max(0))
        pos = pos[perm]
        n = k_keep
        if i < L - 1:
            k = 6 + 2 * i
            sq = np.sum(pos * pos, -1)
            dist = sq[:, None] + sq[None, :] - 2 * (pos @ pos.T)
            np.fill_diagonal(dist, np.inf)
            idx = np.argpartition(dist, k, 1)[:, :k]
            srt = np.take_along_axis(dist, idx, 1).argsort(1, kind="stable")
            idx = np.take_along_axis(idx, srt, 1)
            dst = np.repeat(np.arange(n), k)
            src = idx.reshape(-1)
    return xs


# ----------------------------------------------------------------------------
# kNN host validation
# ----------------------------------------------------------------------------

def _knn_from_cand(cand16, pos, k):
    """cand16: [n, 16] device max-index results (cols sorted by -dist).
    Returns tbl [n, k] of neighbor ids; falls back per-row when needed."""
    n = pos.shape[0]
    selfid = np.arange(n, dtype=np.int64)
    c = cand16.astype(np.int64)
    not_self = c != selfid[:, None]
    # positions of first k non-self entries per row
    cum = np.cumsum(not_self, 1)
    takec = (cum <= k) & not_self
    enough = cum[:, -1] >= k
    tbl = np.zeros((n, k), np.int64)
    rows_ok = np.flatnonzero(enough)
    # fill via argsort trick: order of selected cols preserved
    sel = np.where(takec, np.arange(16)[None, :], 99)
    ordcols = np.argsort(sel, 1, kind="stable")[:, :k]
    tbl = np.take_along_axis(c, ordcols, 1)
    # validity: unique and in range
    srt = np.sort(tbl, 1)
    dup = (srt[:, 1:] == srt[:, :-1]).any(1)
    oob = (tbl < 0).any(1) | (tbl >= n).any(1)
    bad = dup | oob | ~enough
    bad_rows = np.flatnonzero(bad)
    if len(bad_rows):
        sq = np.sum(pos * pos, 1)
        for i in bad_rows:
            d = sq + sq[i] - 2.0 * (pos @ pos[i])
            d[i] = np.inf
            idx = np.argpartition(d, k)[:k]
            tbl[i] = idx[np.argsort(d[idx], kind="stable")]
    return tbl


# ----------------------------------------------------------------------------
# main kernel
# ----------------------------------------------------------------------------

_EXEC_NS = []


def kernel(x, pos, edge_index, conv0_wr, conv0_br, conv0_wl, conv_wr, conv_br,
           conv_wl, pool_lin_w, pool_lin_b, pool_att_w, pool_att_b, le1_w,
           le1_b, le2_w, le3_w, le3_b, lin1_w, lin1_b, lin2_w, lin2_b):
    t_start = time.perf_counter()
    _EXEC_NS.clear()
    x = np.asarray(x, _f32)
    pos = np.asarray(pos, _f32)
    ei = np.asarray(edge_index).astype(np.int64)

    W = {
        "wr": [np.asarray(conv0_wr, _f32)] + [np.asarray(conv_wr[i], _f32)
                                              for i in range(L - 1)],
        "br": [np.asarray(conv0_br, _f32)] + [np.asarray(conv_br[i], _f32)
                                              for i in range(L - 1)],
        "wl": [np.asarray(conv0_wl, _f32)] + [np.asarray(conv_wl[i], _f32)
                                              for i in range(L - 1)],
        "lw": [np.asarray(pool_lin_w[i], _f32) for i in range(L)],
        "lb": [np.asarray(pool_lin_b[i], _f32) for i in range(L)],
        "aw": [np.asarray(pool_att_w[i], _f32) for i in range(L)],
        "ab": [float(pool_att_b[i]) for i in range(L)],
        "l1w": [np.asarray(le1_w[i], _f32) for i in range(L)],
        "l1b": [float(le1_b[i]) for i in range(L)],
        "l2w": [np.asarray(le2_w[i], _f32) for i in range(L)],
        "l3w": [np.asarray(le3_w[i], _f32) for i in range(L)],
        "l3b": [float(le3_b[i]) for i in range(L)],
    }
    try:
        xs = _device_forward(x, pos, ei, W)
    except Exception:
        import traceback
        traceback.print_exc()
        print("kernel: device path failed; numpy fallback")
        xs = _np_reference(x, pos, ei[0], ei[1], W)

    hcat = np.concatenate(xs)[None, :].astype(_f32)
    h1 = np.maximum(hcat @ np.asarray(lin1_w, _f32) +
                    np.asarray(lin1_b, _f32), 0)
    out = (h1 @ np.asarray(lin2_w, _f32) + np.asarray(lin2_b, _f32))
    dt_ns = int((time.perf_counter() - t_start) * 1e9)
    _EXEC_NS.append(("kernel", dt_ns))
    return out.astype(_f32)


def _layer_weights(W, i):
    """Pack per-layer pool/fitness weight vectors for the L programs."""
    lw, lb = W["lw"][i], W["lb"][i]
    aw, ab = W["aw"][i], W["ab"][i]
    wq = (lw @ aw[:HID]).astype(_f32)
    qb = float(lb @ aw[:HID] + ab)
    rep = lambda v: np.ascontiguousarray(np.asarray(v, _f32))[None, :]
    return {
        "br": rep(W["br"][i]),
        "wq": rep(wq),
        "aw2": rep(aw[HID:]),
        "l1w": rep(W["l1w"][i]),
        "l2w": rep(W["l2w"][i]),
        "l3w": rep(W["l3w"][i]),
        "qb": _rep128(qb),
        "l1b": _rep128(W["l1b"][i]),
        "l3b": _rep128(W["l3b"][i]),
    }


def _device_forward(x, pos, ei, W):
    src, dst = ei[0], ei[1]

    # ---------------- layer 0 host prep (pure numpy, overlaps warm) --------
    deg0 = np.bincount(dst, minlength=R0).astype(np.int64)
    D0C = max(int(deg0.max()), 1)
    name0 = "L0_%d" % D0C

    x0 = np.zeros((X0_ROWS, IN_CH), _f32)
    x0[:N0] = x
    SENT0 = R0
    tblC, _ = _slot_table(src, dst, R0, D0C, SENT0)
    cidx0 = _idx_to_i16_tile(_slotmajor_list(tblC))
    tblP = np.concatenate(
        [np.arange(R0, dtype=np.int64)[:, None], tblC], 1)
    tblP[N0:, 0] = SENT0   # pad rows: no self slot
    pidx0 = _idx_to_i16_tile(_slotmajor_list(tblP))
    invdeg0 = (1.0 / np.maximum(deg0, 1.0)).astype(_f32)[:, None]
    cnt0 = (deg0 + 1).astype(_f32)[:, None]
    lw0 = _layer_weights(W, 0)
    wxm = np.zeros((128, HID), _f32)
    wxm[0:IN_CH] = W["wl"][0]
    wxm[IN_CH:2 * IN_CH] = W["wr"][0]

    for th in _BUILD_THREADS:
        th.join()
    L0run = _RUNNERS.get(name0) or _get_runner(name0, lambda: _build_L0(D0C))
    L12run = _RUNNERS.get("L12") or _get_runner("L12", _build_L12)
    Krun = _RUNNERS.get("K") or _get_runner("K", _build_K)

    B = _get_bass()
    jax, jnp = B["jax"], B["jnp"]
    dev = jax.devices()[0]
    put = lambda a: jax.device_put(a, dev)

    in0 = {"x": put(x0), "cidx": put(cidx0), "pidx": put(pidx0),
           "invdeg": put(invdeg0), "cnt": put(cnt0), "wxm": put(wxm)}
    in0.update({k: put(v) for k, v in lw0.items()})
    # queue layer-1/2 weights now; transfers overlap the L0/K launches
    lw_next = {}
    for j in (1, 2):
        d = _layer_weights(W, j)
        d["wr"] = np.ascontiguousarray(
            W["wr"][j].reshape(4, 128, HID).transpose(1, 0, 2))
        d["wl"] = np.ascontiguousarray(
            W["wl"][j].reshape(4, 128, HID).transpose(1, 0, 2))
        d["invdeg"] = _rep128(1.0 / (6 + 2 * (j - 1)))
        d["cnt"] = _rep128(7 + 2 * (j - 1))
        lw_next[j] = {k: put(v) for k, v in d.items()}
    t0 = time.perf_counter()
    r0 = L0run(in0)
    z0 = np.asarray(r0["z"])[:N0, 0]
    _EXEC_NS.append(("L0", int((time.perf_counter() - t0) * 1e9)))

    xs_out = []
    feat_xn = r0["xn"]
    cur_pos = pos
    n_cur = N0
    for i in range(L):
        k_keep = int(math.ceil(RATIO * n_cur))
        z = z0
        # ---- host top-k ----
        perm = np.argpartition(-z, k_keep - 1)[:k_keep]
        fit = (1.0 / (1.0 + np.exp(-z[perm].astype(np.float64)))).astype(_f32)
        sel = np.zeros(R1, np.int64)
        sel[:k_keep] = perm
        fv = np.zeros((R1, 1), _f32)
        fv[:k_keep, 0] = fit
        msk = np.full((R1, 1), -1e30, _f32)
        msk[:k_keep] = 0.0
        cur_pos = cur_pos[perm]
        n_cur = k_keep
        # ---- kNN inputs ----
        if i < L - 1:
            kk = 6 + 2 * i
            sq = np.sum(cur_pos * cur_pos, 1, dtype=_f32)
            qT = np.zeros((4, XT_COLS), _f32)
            qT[0, :n_cur] = 2.0 * cur_pos[:, 0]
            qT[1, :n_cur] = 2.0 * cur_pos[:, 1]
            qT[2, :n_cur] = -1.0
            qT[3, :n_cur] = -sq
            cand = np.zeros((4, XT_COLS), _f32)
            cand[0, :n_cur] = cur_pos[:, 0]
            cand[1, :n_cur] = cur_pos[:, 1]
            cand[2, :n_cur] = sq
            cand[2, n_cur:] = 1e30
            cand[3, :] = 1.0
        else:
            kk = 0
            qT = np.zeros((4, XT_COLS), _f32)
            cand = np.zeros((4, XT_COLS), _f32)
        t0 = time.perf_counter()
        rK = Krun({"xn": feat_xn, "sidx": put(_idx_to_i16_tile(sel)),
                   "fv": put(fv), "msk": put(msk),
                   "qT": put(qT), "cand": put(cand)})
        xs_out.append(rK["xsp"])   # device partial max; reduced at the end
        _EXEC_NS.append(("K%d" % i, int((time.perf_counter() - t0) * 1e9)))
        if i == L - 1:
            break
        cand16 = np.asarray(rK["knn"])[:n_cur]
        tbl = _knn_from_cand(cand16, cur_pos, kk)

        # ---- next layer tables ----
        SENT1 = R1
        tblC1 = np.full((R1, D1C), SENT1, np.int64)
        tblC1[:n_cur, :kk] = tbl
        cidx1 = _idx_to_i16_tile(_slotmajor_list(tblC1))
        tblP1 = np.concatenate(
            [np.arange(R1, dtype=np.int64)[:, None], tblC1], 1)
        tblP1[n_cur:, 0] = SENT1
        pidx1 = _idx_to_i16_tile(_slotmajor_list(tblP1))
        inL = {"x": rK["xo"], "xT": rK["xT"],
               "cidx": put(cidx1), "pidx": put(pidx1)}
        inL.update(lw_next[i + 1])
        t0 = time.perf_counter()
        rL = L12run(inL)
        z0 = np.asarray(rL["z"])[:n_cur, 0]
        _EXEC_NS.append(("L%d" % (i + 1),
                         int((time.perf_counter() - t0) * 1e9)))
        feat_xn = rL["xn"]
    return [np.asarray(p).max(0) for p in xs_out]


def total_exec_ns():
    return sum(v for k, v in _EXEC_NS if k == "kernel")


def exec_breakdown():
    return list(_EXEC_NS)


# revision 13
# speedup vs baseline: 1.0852x; 1.0852x over previous
"""ASAP-GNN classifier on trn2 via Bass/Tile.

Architecture (v2): single NeuronCore, device-resident features between
launches. Three compiled programs (NEFFs), built/compiled at import time in
background threads:

  L0  : layer-0 GraphConv + ASAPool attention + LEConv fitness over the
        irregular input graph (slot-table gathers, For_i loops over 157
        row-blocks of 128 nodes).
  L12 : same pipeline for layers 1 and 2 over the fixed-degree kNN graphs
        (shared program; layer-2's 5000 nodes padded to layer-1's shape).
  K   : top-half "select" (gather xn[perm]*fv -> next x + transposed copy +
        running global max) fused with the dense kNN distance scan
        (max8/max_index, two rounds -> 16 neighbor candidates).

Host does only: slot-table construction, top-k via argpartition on the
fitness logits, kNN candidate validation, and the final 1x1536 MLP. Per
layer one launch round-trip for fitness -> perm and one for select+kNN:
6 launches total, ~KBs of traffic each after the initial ~17MB upload.
"""

import math
import threading
import time
import numpy as np

N0 = 20000
IN_CH = 64
HID = 512
OUT = 10
L = 3
RATIO = 0.5

_f32 = np.float32

# ---- geometry constants (hardcoded; program shapes) ----
NB0 = 157                   # layer-0 row blocks
R0 = NB0 * 128              # 20096
X0_ROWS = R0 + 128          # feat_x0 rows (sentinel row = R0, zeros)
D0C_DEFAULT = 17            # layer-0 max in-degree (rebuilt if actual differs)

NB1 = 79                    # layer-1/2 row blocks
R1 = NB1 * 128              # 10112
D1C = 8                     # conv slots for kNN layers (k<=8)
D1P = 9                     # pool slots (self + 8)

XN_ROWS = 20352             # unified xn/x buffer rows (>= R0 + sentinel)
XT_COLS = 10240             # x1T columns (>= R1)
NCH = XT_COLS // 512        # kNN candidate chunks (20)


# ----------------------------------------------------------------------------
# bass plumbing
# ----------------------------------------------------------------------------

_BASS = {}


def _get_bass():
    if not _BASS:
        import concourse.bass as bass
        import concourse.bacc as bacc
        import concourse.mybir as mybir
        from concourse.tile import TileContext
        from concourse.masks import make_identity
        from concourse.bass import ds
        from concourse import bass2jax
        import jax
        import jax.numpy as jnp
        bass2jax.install_neuronx_cc_hook()
        _BASS.update(bass=bass, bacc=bacc, mybir=mybir, TileContext=TileContext,
                     make_identity=make_identity, ds=ds, bass2jax=bass2jax,
                     jax=jax, jnp=jnp)
    return _BASS


class _Launcher:
    """Compiled 1-core bass program; inputs/outputs stay jax device arrays."""

    def __init__(self, nc):
        B = _get_bass()
        jax, jnp, mybir = B["jax"], B["jnp"], B["mybir"]
        bass2jax = B["bass2jax"]
        partition_name = (nc.partition_id_tensor.name
                          if nc.partition_id_tensor else None)
        in_names, in_avals, out_names, out_avals = [], [], [], []
        for alloc in nc.m.functions[0].allocations:
            if not isinstance(alloc, mybir.MemoryLocationSet):
                continue
            name = alloc.memorylocations[0].name
            if alloc.kind == "ExternalInput":
                if name != partition_name:
                    in_names.append(name)
                    in_avals.append(jax.ShapeDtypeStruct(
                        tuple(alloc.tensor_shape), mybir.dt.np(alloc.dtype)))
            elif alloc.kind == "ExternalOutput":
                out_names.append(name)
                out_avals.append(jax.core.ShapedArray(
                    tuple(alloc.tensor_shape), mybir.dt.np(alloc.dtype)))
        self.in_names = in_names
        self.in_avals = in_avals
        self.out_names = out_names
        self.out_avals = out_avals
        n_params = len(in_names)
        all_names = in_names + out_names + (
            [partition_name] if partition_name else [])
        donate = tuple(range(n_params, n_params + len(out_names)))

        def _body(*args):
            operands = list(args)
            if partition_name is not None:
                operands.append(bass2jax.partition_id_tensor())
            outs = bass2jax._bass_exec_p.bind(
                *operands, out_avals=tuple(out_avals),
                in_names=tuple(all_names), out_names=tuple(out_names),
                lowering_input_output_aliases=(),
                sim_require_finite=True, sim_require_nnan=True, nc=nc)
            return tuple(outs)

        self._jit = jax.jit(_body, donate_argnums=donate, keep_unused=True)
        self._compiled = None

    def warm(self):
        """AOT-compile the executable (no execution)."""
        B = _get_bass()
        jax = B["jax"]
        out_structs = [jax.ShapeDtypeStruct(av.shape, av.dtype)
                       for av in self.out_avals]
        self._compiled = self._jit.lower(*self.in_avals,
                                         *out_structs).compile()
        return self

    def __call__(self, in_map):
        B = _get_bass()
        jnp = B["jnp"]
        args = [in_map[nm] for nm in self.in_names]
        zeros = [jnp.zeros(av.shape, av.dtype) for av in self.out_avals]
        fn = self._compiled if self._compiled is not None else self._jit
        outs = fn(*args, *zeros)
        return dict(zip(self.out_names, outs))


# ----------------------------------------------------------------------------
# host helpers
# ----------------------------------------------------------------------------

def _idx_to_i16_tile(idx_list):
    """Compact dma_gather idx tile [16, S]: element m -> partition m%16,
    col m//16. Replicated across the 8 Q7 groups on device."""
    n = len(idx_list)
    S = (n + 15) // 16
    a = np.full((S, 16), -1, np.int16)
    a.reshape(-1)[:n] = idx_list.astype(np.int16)
    return np.ascontiguousarray(a.T)


def _slot_table(src, dst, nrows, D, sentinel):
    """[nrows, D] slot table: row i lists srcs of i's in-edges, sentinel pad."""
    deg = np.bincount(dst, minlength=nrows).astype(np.int64)
    order = np.argsort(dst, kind="stable")
    ss = src[order]
    dsrt = dst[order]
    starts = np.zeros(nrows + 1, np.int64)
    np.cumsum(deg, out=starts[1:])
    slot = np.arange(len(dsrt)) - starts[dsrt]
    tbl = np.full((nrows, D), sentinel, np.int64)
    tbl[dsrt, slot] = ss
    return tbl, deg


def _slotmajor_list(tbl):
    """[rows, D] -> block-slot-major gather list (per 128-block, slot-major)."""
    rows, D = tbl.shape
    nb = rows // 128
    return np.ascontiguousarray(
        tbl.reshape(nb, 128, D).transpose(0, 2, 1)).reshape(-1)


def _rep128(v):
    return np.full((128, 1), v, _f32)


# ----------------------------------------------------------------------------
# program builders
# ----------------------------------------------------------------------------

def _tree_sum(nc, g, n, view):
    """In-place binary-tree reduce over slot axis: view(g, lo, cnt) -> AP.
    Result lands in slot 0. Returns nothing."""
    w = n
    while w > 1:
        h = w // 2
        nc.vector.tensor_add(view(0, h), view(0, h), view(h, h))
        if w % 2:
            nc.vector.tensor_add(view(0, 1), view(0, 1), view(w - 1, 1))
        w = h


def _tree_max(nc, out_t, g, n, gview, oview):
    """Max over n slots of g into out_t (slot tile of n//2 width)."""
    h = n // 2
    nc.vector.tensor_max(oview(0, h), gview(0, h), gview(h, h))
    if n % 2:
        nc.vector.tensor_max(oview(0, 1), oview(0, 1), gview(n - 1, 1))
    w = h
    while w > 1:
        h2 = w // 2
        nc.vector.tensor_max(oview(0, h2), oview(0, h2), oview(h2, h2))
        if w % 2:
            nc.vector.tensor_max(oview(0, 1), oview(0, 1), oview(w - 1, 1))
        w = h2


def _build_L0(D0C):
    """Layer-0: conv + pool + fitness over irregular graph."""
    B = _get_bass()
    bacc, mybir, TileContext = B["bacc"], B["mybir"], B["TileContext"]
    ds, make_identity = B["ds"], B["make_identity"]
    dt = mybir.dt
    D0P = D0C + 1
    SC = NB0 * ((128 * D0C) // 16)       # conv idx tile cols
    SP = NB0 * ((128 * D0P) // 16)       # pool idx tile cols
    F = IN_CH
    HROWS = R0 + 128                     # feat_h rows, sentinel = R0

    nc = bacc.Bacc("TRN2", target_bir_lowering=False)
    x_d = nc.dram_tensor("x", [X0_ROWS, F], dt.float32, kind="ExternalInput")
    cidx_d = nc.dram_tensor("cidx", [16, SC], dt.int16, kind="ExternalInput")
    pidx_d = nc.dram_tensor("pidx", [16, SP], dt.int16, kind="ExternalInput")
    invdeg_d = nc.dram_tensor("invdeg", [R0, 1], dt.float32, kind="ExternalInput")
    cnt_d = nc.dram_tensor("cnt", [R0, 1], dt.float32, kind="ExternalInput")
    wxm_d = nc.dram_tensor("wxm", [128, HID], dt.float32, kind="ExternalInput")
    br_d = nc.dram_tensor("br", [1, HID], dt.float32, kind="ExternalInput")
    wq_d = nc.dram_tensor("wq", [1, HID], dt.float32, kind="ExternalInput")
    aw2_d = nc.dram_tensor("aw2", [1, HID], dt.float32, kind="ExternalInput")
    l1w_d = nc.dram_tensor("l1w", [1, HID], dt.float32, kind="ExternalInput")
    l2w_d = nc.dram_tensor("l2w", [1, HID], dt.float32, kind="ExternalInput")
    l3w_d = nc.dram_tensor("l3w", [1, HID], dt.float32, kind="ExternalInput")
    qb_d = nc.dram_tensor("qb", [128, 1], dt.float32, kind="ExternalInput")
    l1b_d = nc.dram_tensor("l1b", [128, 1], dt.float32, kind="ExternalInput")
    l3b_d = nc.dram_tensor("l3b", [128, 1], dt.float32, kind="ExternalInput")

    h_d = nc.dram_tensor("fh", [HROWS, 576], dt.float32, kind="Internal")
    a_d = nc.dram_tensor("fa", [HROWS, 64], dt.float32, kind="Internal")
    zb_d = nc.dram_tensor("zb", [R0, 1], dt.float32, kind="Internal")
    xn_d = nc.dram_tensor("xn", [XN_ROWS, HID], dt.float32,
                          kind="ExternalOutput")
    z_d = nc.dram_tensor("z", [R0, 1], dt.float32, kind="ExternalOutput")

    with TileContext(nc) as tc:
        with (
            tc.tile_pool(name="const", bufs=1) as cpool,
            tc.tile_pool(name="ps", bufs=2, space="PSUM") as pspool,
        ):
            ident = cpool.tile([128, 128], dt.float32)
            make_identity(nc, ident[:])
            wxm_sb = cpool.tile([128, HID], dt.float32)
            nc.sync.dma_start(wxm_sb[:], wxm_d[:, :])
            br_sb = cpool.tile([128, HID], dt.float32)
            nc.sync.dma_start(br_sb[:], br_d[0:1, :].to_broadcast([128, HID]))
            wq_sb = cpool.tile([128, HID], dt.float32)
            nc.sync.dma_start(wq_sb[:], wq_d[0:1, :].to_broadcast([128, HID]))
            aw2_sb = cpool.tile([128, HID], dt.float32)
            nc.sync.dma_start(aw2_sb[:], aw2_d[0:1, :].to_broadcast([128, HID]))
            l1w_sb = cpool.tile([128, HID], dt.float32)
            nc.sync.dma_start(l1w_sb[:], l1w_d[0:1, :].to_broadcast([128, HID]))
            l2w_sb = cpool.tile([128, HID], dt.float32)
            nc.sync.dma_start(l2w_sb[:], l2w_d[0:1, :].to_broadcast([128, HID]))
            l3w_sb = cpool.tile([128, HID], dt.float32)
            nc.sync.dma_start(l3w_sb[:], l3w_d[0:1, :].to_broadcast([128, HID]))
            qb_sb = cpool.tile([128, 1], dt.float32)
            nc.sync.dma_start(qb_sb[:], qb_d[:, :])
            l1b_sb = cpool.tile([128, 1], dt.float32)
            nc.sync.dma_start(l1b_sb[:], l1b_d[:, :])
            l3b_sb = cpool.tile([128, 1], dt.float32)
            nc.sync.dma_start(l3b_sb[:], l3b_d[:, :])
            pidx_sb = cpool.tile([128, SP], dt.int16)
            for _g in range(8):
                nc.sync.dma_start(pidx_sb[_g * 16:(_g + 1) * 16, :],
                                  pidx_d[:, :])
            # sentinel rows: feat_h[R0] = zeros except js col = -1e30;
            # feat_a[R0] = 0
            srow = cpool.tile([1, 576], dt.float32)
            nc.vector.memset(srow[:], 0.0)
            nc.vector.memset(srow[:, 512:513], -1e30)
            nc.sync.dma_start(h_d[R0:R0 + 1, :], srow[:])
            nc.sync.dma_start(a_d[R0:R0 + 1, :], srow[:, 0:64])

            # ---- phase A: conv ----
            SCB = (128 * D0C) // 16
            with tc.tile_pool(name="conv", bufs=2) as wp:
                cidx_sb = wp.tile([128, SC], dt.int16, tag="cidx", bufs=1)
                for _g in range(8):
                    nc.sync.dma_start(cidx_sb[_g * 16:(_g + 1) * 16, :],
                                      cidx_d[:, :])

                def conv_body(i):
                    g = wp.tile([128, D0C, F], dt.float32, tag="g")
                    nc.gpsimd.dma_gather(
                        out_ap=g[:], in_ap=x_d[:, :],
                        idxs_ap=cidx_sb[:, ds(i * SCB, SCB)],
                        num_idxs=128 * D0C, num_idxs_reg=128 * D0C,
                        elem_size=F, single_packet=False)
                    _tree_sum(nc, g, D0C,
                              lambda lo, cnt: g[:, lo:lo + cnt, :])
                    iv = wp.tile([128, 1], dt.float32, tag="iv")
                    nc.sync.dma_start(iv[:], invdeg_d[ds(i * 128, 128), :])
                    xm = wp.tile([128, 128], dt.float32, tag="xm")
                    nc.sync.dma_start(xm[:, 0:F], x_d[ds(i * 128, 128), :])
                    nc.vector.tensor_scalar_mul(xm[:, F:2 * F], g[:, 0, :],
                                                iv[:])
                    tp = pspool.tile([128, 128], dt.float32, tag="tp")
                    nc.tensor.transpose(tp[:], xm[:], ident[:])
                    lhsT = wp.tile([128, 128], dt.float32, tag="lhsT")
                    nc.vector.tensor_copy(lhsT[:], tp[:])
                    hps = pspool.tile([128, HID], dt.float32, tag="hps")
                    nc.tensor.matmul(hps[:], lhsT[:], wxm_sb[:],
                                     start=True, stop=True)
                    hsb = wp.tile([128, 576], dt.float32, tag="hsb")
                    nc.vector.tensor_add(
                        hsb[:, 0:HID], hps[:],
                        br_sb[:])
                    nc.vector.tensor_scalar_max(hsb[:, 0:HID], hsb[:, 0:HID],
                                                0.0)
                    tmp = wp.tile([128, HID], dt.float32, tag="tmp")
                    nc.vector.tensor_mul(tmp[:], hsb[:, 0:HID],
                                         aw2_sb[:])
                    nc.vector.tensor_reduce(hsb[:, 512:513], tmp[:],
                                            axis=mybir.AxisListType.X,
                                            op=mybir.AluOpType.add)
                    nc.sync.dma_start(h_d[ds(i * 128, 128), 0:513],
                                      hsb[:, 0:513])
                tc.For_i_unrolled(0, NB0, 1, conv_body, max_unroll=2)

            # ---- phase B: pool ----
            SPB = (128 * D0P) // 16
            with tc.tile_pool(name="pool", bufs=2) as wp:
                def pool_body(i):
                    g = wp.tile([128, D0P, 576], dt.float32, tag="g")
                    nc.gpsimd.dma_gather(
                        out_ap=g[:], in_ap=h_d[:, :],
                        idxs_ap=pidx_sb[:, ds(i * SPB, SPB)],
                        num_idxs=128 * D0P, num_idxs_reg=128 * D0P,
                        elem_size=576, single_packet=False)
                    xq = wp.tile([128, D0P // 2, HID], dt.float32, tag="xq")
                    _tree_max(nc, xq, g, D0P,
                              lambda lo, cnt: g[:, lo:lo + cnt, 0:HID],
                              lambda lo, cnt: xq[:, lo:lo + cnt, :])
                    tmp = wp.tile([128, HID], dt.float32, tag="tmp")
                    nc.vector.tensor_mul(tmp[:], xq[:, 0, :],
                                         wq_sb[:])
                    qs = wp.tile([128, 1], dt.float32, tag="qs")
                    nc.vector.tensor_reduce(qs[:], tmp[:],
                                            axis=mybir.AxisListType.X,
                                            op=mybir.AluOpType.add)
                    nc.vector.tensor_add(qs[:], qs[:], qb_sb[:])
                    # score = leaky_relu(qs + js)
                    sc = wp.tile([128, D0P], dt.float32, tag="sc")
                    jsv = g[:, :, 512:513].squeeze(2)
                    nc.vector.tensor_scalar_add(sc[:], jsv, qs[:])
                    sc2 = wp.tile([128, D0P], dt.float32, tag="sc2")
                    nc.vector.tensor_scalar_mul(sc2[:], sc[:], 0.2)
                    nc.vector.tensor_max(sc[:], sc[:], sc2[:])
                    m = wp.tile([128, 1], dt.float32, tag="m")
                    nc.vector.tensor_reduce(m[:], sc[:],
                                            axis=mybir.AxisListType.X,
                                            op=mybir.AluOpType.max)
                    nc.vector.tensor_scalar(sc[:], sc[:], m[:], None,
                                            op0=mybir.AluOpType.subtract)
                    nc.scalar.activation(sc[:], sc[:],
                                         mybir.ActivationFunctionType.Exp)
                    ssum = wp.tile([128, 1], dt.float32, tag="ssum")
                    nc.vector.tensor_reduce(ssum[:], sc[:],
                                            axis=mybir.AxisListType.X,
                                            op=mybir.AluOpType.add)
                    rec = wp.tile([128, 1], dt.float32, tag="rec")
                    nc.vector.reciprocal(rec[:], ssum[:])
                    nc.vector.tensor_scalar_mul(sc[:], sc[:], rec[:])
                    # xn = sum_s att_s * h_s  (scale slots in place, tree add)
                    gh = g[:, :, 0:HID]
                    nc.vector.tensor_mul(
                        gh, gh, sc[:].unsqueeze(2).to_broadcast(
                            [128, D0P, HID]))
                    _tree_sum(nc, g, D0P,
                              lambda lo, cnt: g[:, lo:lo + cnt, 0:HID])
                    xn = g[:, 0, 0:HID]
                    nc.sync.dma_start(xn_d[ds(i * 128, 128), :], xn)
                    # fitness scalars
                    nc.vector.tensor_mul(tmp[:], xn,
                                         l1w_sb[:])
                    av = wp.tile([128, 1], dt.float32, tag="av")
                    nc.vector.tensor_reduce(av[:], tmp[:],
                                            axis=mybir.AxisListType.X,
                                            op=mybir.AluOpType.add)
                    nc.sync.dma_start(a_d[ds(i * 128, 128), 0:1], av[:])
                    nc.vector.tensor_mul(tmp[:], xn,
                                         l2w_sb[:])
                    bv = wp.tile([128, 1], dt.float32, tag="bv")
                    nc.vector.tensor_reduce(bv[:], tmp[:],
                                            axis=mybir.AxisListType.X,
                                            op=mybir.AluOpType.add)
                    nc.vector.tensor_mul(tmp[:], xn,
                                         l3w_sb[:])
                    cv = wp.tile([128, 1], dt.float32, tag="cv")
                    nc.vector.tensor_reduce(cv[:], tmp[:],
                                            axis=mybir.AxisListType.X,
                                            op=mybir.AluOpType.add)
                    ct = wp.tile([128, 1], dt.float32, tag="ct")
                    nc.sync.dma_start(ct[:], cnt_d[ds(i * 128, 128), :])
                    # zb = c + l3b - cnt*b + cnt*l1b
                    zb = wp.tile([128, 1], dt.float32, tag="zb")
                    nc.vector.tensor_mul(zb[:], ct[:], bv[:])
                    nc.vector.tensor_sub(zb[:], cv[:], zb[:])
                    nc.vector.tensor_add(zb[:], zb[:], l3b_sb[:])
                    lb1 = wp.tile([128, 1], dt.float32, tag="lb1")
                    nc.vector.tensor_mul(lb1[:], ct[:], l1b_sb[:])
                    nc.vector.tensor_add(zb[:], zb[:], lb1[:])
                    nc.sync.dma_start(zb_d[ds(i * 128, 128), :], zb[:])
                tc.For_i_unrolled(0, NB0, 1, pool_body, max_unroll=2)

            # ---- phase C: fitness gather ----
            with tc.tile_pool(name="fit", bufs=2) as wp:
                def fit_body(i):
                    ga = wp.tile([128, D0P, 64], dt.float32, tag="ga")
                    nc.gpsimd.dma_gather(
                        out_ap=ga[:], in_ap=a_d[:, :],
                        idxs_ap=pidx_sb[:, ds(i * SPB, SPB)],
                        num_idxs=128 * D0P, num_idxs_reg=128 * D0P,
                        elem_size=64, single_packet=False)
                    zs = wp.tile([128, 1], dt.float32, tag="zs")
                    nc.vector.tensor_reduce(zs[:], ga[:, :, 0:1].squeeze(2),
                                            axis=mybir.AxisListType.X,
                                            op=mybir.AluOpType.add)
                    zb = wp.tile([128, 1], dt.float32, tag="zb2")
                    nc.sync.dma_start(zb[:], zb_d[ds(i * 128, 128), :])
                    nc.vector.tensor_add(zs[:], zs[:], zb[:])
                    nc.sync.dma_start(z_d[ds(i * 128, 128), :], zs[:])
                tc.For_i_unrolled(0, NB0, 1, fit_body, max_unroll=4)
    nc.compile()
    return nc


def _build_L12():
    """Layers 1/2: conv + pool + fitness over fixed-degree kNN graph."""
    B = _get_bass()
    bacc, mybir, TileContext = B["bacc"], B["mybir"], B["TileContext"]
    ds, make_identity = B["ds"], B["make_identity"]
    dt = mybir.dt
    F = HID
    SC = NB1 * ((128 * D1C) // 16)
    SP = NB1 * ((128 * D1P) // 16)
    HROWS = R1 + 128                    # sentinel = R1

    nc = bacc.Bacc("TRN2", target_bir_lowering=False)
    x_d = nc.dram_tensor("x", [XN_ROWS, F], dt.float32, kind="ExternalInput")
    xT_d = nc.dram_tensor("xT", [F, XT_COLS], dt.float32, kind="ExternalInput")
    cidx_d = nc.dram_tensor("cidx", [16, SC], dt.int16, kind="ExternalInput")
    pidx_d = nc.dram_tensor("pidx", [16, SP], dt.int16, kind="ExternalInput")
    invdeg_d = nc.dram_tensor("invdeg", [128, 1], dt.float32,
                              kind="ExternalInput")
    cnt_d = nc.dram_tensor("cnt", [128, 1], dt.float32, kind="ExternalInput")
    wr_d = nc.dram_tensor("wr", [128, 4, HID], dt.float32,
                          kind="ExternalInput")
    wl_d = nc.dram_tensor("wl", [128, 4, HID], dt.float32,
                          kind="ExternalInput")
    br_d = nc.dram_tensor("br", [1, HID], dt.float32, kind="ExternalInput")
    wq_d = nc.dram_tensor("wq", [1, HID], dt.float32, kind="ExternalInput")
    aw2_d = nc.dram_tensor("aw2", [1, HID], dt.float32, kind="ExternalInput")
    l1w_d = nc.dram_tensor("l1w", [1, HID], dt.float32, kind="ExternalInput")
    l2w_d = nc.dram_tensor("l2w", [1, HID], dt.float32, kind="ExternalInput")
    l3w_d = nc.dram_tensor("l3w", [1, HID], dt.float32, kind="ExternalInput")
    qb_d = nc.dram_tensor("qb", [128, 1], dt.float32, kind="ExternalInput")
    l1b_d = nc.dram_tensor("l1b", [128, 1], dt.float32, kind="ExternalInput")
    l3b_d = nc.dram_tensor("l3b", [128, 1], dt.float32, kind="ExternalInput")

    h_d = nc.dram_tensor("fh", [HROWS, 576], dt.float32, kind="Internal")
    a_d = nc.dram_tensor("fa", [HROWS, 64], dt.float32, kind="Internal")
    zb_d = nc.dram_tensor("zb", [R1, 1], dt.float32, kind="Internal")
    xn_d = nc.dram_tensor("xn", [XN_ROWS, HID], dt.float32,
                          kind="ExternalOutput")
    z_d = nc.dram_tensor("z", [R1, 1], dt.float32, kind="ExternalOutput")

    with TileContext(nc) as tc:
        with (
            tc.tile_pool(name="const", bufs=1) as cpool,
            tc.tile_pool(name="ps", bufs=2, space="PSUM") as pspool,
        ):
            ident = cpool.tile([128, 128], dt.float32)
            make_identity(nc, ident[:])
            wr_sb = cpool.tile([128, 4, HID], dt.float32)
            nc.sync.dma_start(wr_sb[:], wr_d[:, :, :])
            wl_sb = cpool.tile([128, 4, HID], dt.float32)
            nc.sync.dma_start(wl_sb[:], wl_d[:, :, :])
            br_sb = cpool.tile([128, HID], dt.float32)
            nc.sync.dma_start(br_sb[:], br_d[0:1, :].to_broadcast([128, HID]))
            wq_sb = cpool.tile([128, HID], dt.float32)
            nc.sync.dma_start(wq_sb[:], wq_d[0:1, :].to_broadcast([128, HID]))
            aw2_sb = cpool.tile([128, HID], dt.float32)
            nc.sync.dma_start(aw2_sb[:], aw2_d[0:1, :].to_broadcast([128, HID]))
            l1w_sb = cpool.tile([128, HID], dt.float32)
            nc.sync.dma_start(l1w_sb[:], l1w_d[0:1, :].to_broadcast([128, HID]))
            l2w_sb = cpool.tile([128, HID], dt.float32)
            nc.sync.dma_start(l2w_sb[:], l2w_d[0:1, :].to_broadcast([128, HID]))
            l3w_sb = cpool.tile([128, HID], dt.float32)
            nc.sync.dma_start(l3w_sb[:], l3w_d[0:1, :].to_broadcast([128, HID]))
            qb_sb = cpool.tile([128, 1], dt.float32)
            nc.sync.dma_start(qb_sb[:], qb_d[:, :])
            l1b_sb = cpool.tile([128, 1], dt.float32)
            nc.sync.dma_start(l1b_sb[:], l1b_d[:, :])
            l3b_sb = cpool.tile([128, 1], dt.float32)
            nc.sync.dma_start(l3b_sb[:], l3b_d[:, :])
            iv_sb = cpool.tile([128, 1], dt.float32)
            nc.sync.dma_start(iv_sb[:], invdeg_d[:, :])
            ct_sb = cpool.tile([128, 1], dt.float32)
            nc.sync.dma_start(ct_sb[:], cnt_d[:, :])
            cidx_sb = cpool.tile([128, SC], dt.int16)
            for _g in range(8):
                nc.sync.dma_start(cidx_sb[_g * 16:(_g + 1) * 16, :],
                                  cidx_d[:, :])
            pidx_sb = cpool.tile([128, SP], dt.int16)
            for _g in range(8):
                nc.sync.dma_start(pidx_sb[_g * 16:(_g + 1) * 16, :],
                                  pidx_d[:, :])
            srow = cpool.tile([1, 576], dt.float32)
            nc.vector.memset(srow[:], 0.0)
            nc.vector.memset(srow[:, 512:513], -1e30)
            nc.sync.dma_start(h_d[R1:R1 + 1, :], srow[:])
            nc.sync.dma_start(a_d[R1:R1 + 1, :], srow[:, 0:64])

            SCB = (128 * D1C) // 16
            SPB = (128 * D1P) // 16
            with tc.tile_pool(name="conv", bufs=2) as wp:
                def conv_body(i):
                    g = wp.tile([128, D1C, F], dt.float32, tag="g")
                    nc.gpsimd.dma_gather(
                        out_ap=g[:], in_ap=x_d[:, :],
                        idxs_ap=cidx_sb[:, ds(i * SCB, SCB)],
                        num_idxs=128 * D1C, num_idxs_reg=128 * D1C,
                        elem_size=F, single_packet=False)
                    _tree_sum(nc, g, D1C,
                              lambda lo, cnt: g[:, lo:lo + cnt, :])
                    mean = wp.tile([128, F], dt.float32, tag="mean")
                    nc.vector.tensor_scalar_mul(mean[:], g[:, 0, :], iv_sb[:])
                    hps = pspool.tile([128, HID], dt.float32, tag="hps")
                    xt = wp.tile([128, 4, 128], dt.float32, tag="xt")
                    nc.sync.dma_start(
                        xt[:], xT_d[:, ds(i * 128, 128)].rearrange(
                            "(c r) m -> r c m", c=4))
                    mt = wp.tile([128, 4, 128], dt.float32, tag="mt")
                    for c in range(4):
                        tp = pspool.tile([128, 128], dt.float32, tag="tp")
                        nc.tensor.transpose(tp[:],
                                            mean[:, c * 128:(c + 1) * 128],
                                            ident[:])
                        nc.vector.tensor_copy(mt[:, c, :], tp[:])
                    for c in range(4):
                        nc.tensor.matmul(hps[:], xt[:, c, :], wl_sb[:, c, :],
                                         start=(c == 0), stop=False)
                    for c in range(4):
                        nc.tensor.matmul(hps[:], mt[:, c, :], wr_sb[:, c, :],
                                         start=False, stop=(c == 3))
                    hsb = wp.tile([128, 576], dt.float32, tag="hsb")
                    nc.vector.tensor_add(
                        hsb[:, 0:HID], hps[:],
                        br_sb[:])
                    nc.vector.tensor_scalar_max(hsb[:, 0:HID], hsb[:, 0:HID],
                                                0.0)
                    tmp = wp.tile([128, HID], dt.float32, tag="tmp")
                    nc.vector.tensor_mul(tmp[:], hsb[:, 0:HID],
                                         aw2_sb[:])
                    nc.vector.tensor_reduce(hsb[:, 512:513], tmp[:],
                                            axis=mybir.AxisListType.X,
                                            op=mybir.AluOpType.add)
                    nc.sync.dma_start(h_d[ds(i * 128, 128), 0:513],
                                      hsb[:, 0:513])
                tc.For_i_unrolled(0, NB1, 1, conv_body, max_unroll=2)

            with tc.tile_pool(name="pool", bufs=2) as wp:
                def pool_body(i):
                    g = wp.tile([128, D1P, 576], dt.float32, tag="g")
                    nc.gpsimd.dma_gather(
                        out_ap=g[:], in_ap=h_d[:, :],
                        idxs_ap=pidx_sb[:, ds(i * SPB, SPB)],
                        num_idxs=128 * D1P, num_idxs_reg=128 * D1P,
                        elem_size=576, single_packet=False)
                    xq = wp.tile([128, D1P // 2, HID], dt.float32, tag="xq")
                    _tree_max(nc, xq, g, D1P,
                              lambda lo, cnt: g[:, lo:lo + cnt, 0:HID],
                              lambda lo, cnt: xq[:, lo:lo + cnt, :])
                    tmp = wp.tile([128, HID], dt.float32, tag="tmp")
                    nc.vector.tensor_mul(tmp[:], xq[:, 0, :],
                                         wq_sb[:])
                    qs = wp.tile([128, 1], dt.float32, tag="qs")
                    nc.vector.tensor_reduce(qs[:], tmp[:],
                                            axis=mybir.AxisListType.X,
                                            op=mybir.AluOpType.add)
                    nc.vector.tensor_add(qs[:], qs[:], qb_sb[:])
                    sc = wp.tile([128, D1P], dt.float32, tag="sc")
                    jsv = g[:, :, 512:513].squeeze(2)
                    nc.vector.tensor_scalar_add(sc[:], jsv, qs[:])
                    sc2 = wp.tile([128, D1P], dt.float32, tag="sc2")
                    nc.vector.tensor_scalar_mul(sc2[:], sc[:], 0.2)
                    nc.vector.tensor_max(sc[:], sc[:], sc2[:])
                    m = wp.tile([128, 1], dt.float32, tag="m")
                    nc.vector.tensor_reduce(m[:], sc[:],
                                            axis=mybir.AxisListType.X,
                                            op=mybir.AluOpType.max)
                    nc.vector.tensor_scalar(sc[:], sc[:], m[:], None,
                                            op0=mybir.AluOpType.subtract)
                    nc.scalar.activation(sc[:], sc[:],
                                         mybir.ActivationFunctionType.Exp)
                    ssum = wp.tile([128, 1], dt.float32, tag="ssum")
                    nc.vector.tensor_reduce(ssum[:], sc[:],
                                            axis=mybir.AxisListType.X,
                                            op=mybir.AluOpType.add)
                    rec = wp.tile([128, 1], dt.float32, tag="rec")
                    nc.vector.reciprocal(rec[:], ssum[:])
                    nc.vector.tensor_scalar_mul(sc[:], sc[:], rec[:])
                    gh = g[:, :, 0:HID]
                    nc.vector.tensor_mul(
                        gh, gh, sc[:].unsqueeze(2).to_broadcast(
                            [128, D1P, HID]))
                    _tree_sum(nc, g, D1P,
                              lambda lo, cnt: g[:, lo:lo + cnt, 0:HID])
                    xn = g[:, 0, 0:HID]
                    nc.sync.dma_start(xn_d[ds(i * 128, 128), :], xn)
                    nc.vector.tensor_mul(tmp[:], xn,
                                         l1w_sb[:])
                    av = wp.tile([128, 1], dt.float32, tag="av")
                    nc.vector.tensor_reduce(av[:], tmp[:],
                                            axis=mybir.AxisListType.X,
                                            op=mybir.AluOpType.add)
                    nc.sync.dma_start(a_d[ds(i * 128, 128), 0:1], av[:])
                    nc.vector.tensor_mul(tmp[:], xn,
                                         l2w_sb[:])
                    bv = wp.tile([128, 1], dt.float32, tag="bv")
                    nc.vector.tensor_reduce(bv[:], tmp[:],
                                            axis=mybir.AxisListType.X,
                                            op=mybir.AluOpType.add)
                    nc.vector.tensor_mul(tmp[:], xn,
                                         l3w_sb[:])
                    cv = wp.tile([128, 1], dt.float32, tag="cv")
                    nc.vector.tensor_reduce(cv[:], tmp[:],
                                            axis=mybir.AxisListType.X,
                                            op=mybir.AluOpType.add)
                    zb = wp.tile([128, 1], dt.float32, tag="zb")
                    nc.vector.tensor_mul(zb[:], ct_sb[:], bv[:])
                    nc.vector.tensor_sub(zb[:], cv[:], zb[:])
                    nc.vector.tensor_add(zb[:], zb[:], l3b_sb[:])
                    lb1 = wp.tile([128, 1], dt.float32, tag="lb1")
                    nc.vector.tensor_mul(lb1[:], ct_sb[:], l1b_sb[:])
                    nc.vector.tensor_add(zb[:], zb[:], lb1[:])
                    nc.sync.dma_start(zb_d[ds(i * 128, 128), :], zb[:])
                tc.For_i_unrolled(0, NB1, 1, pool_body, max_unroll=2)

            with tc.tile_pool(name="fit", bufs=2) as wp:
                def fit_body(i):
                    ga = wp.tile([128, D1P, 64], dt.float32, tag="ga")
                    nc.gpsimd.dma_gather(
                        out_ap=ga[:], in_ap=a_d[:, :],
                        idxs_ap=pidx_sb[:, ds(i * SPB, SPB)],
                        num_idxs=128 * D1P, num_idxs_reg=128 * D1P,
                        elem_size=64, single_packet=False)
                    zs = wp.tile([128, 1], dt.float32, tag="zs")
                    nc.vector.tensor_reduce(zs[:], ga[:, :, 0:1].squeeze(2),
                                            axis=mybir.AxisListType.X,
                                            op=mybir.AluOpType.add)
                    zb = wp.tile([128, 1], dt.float32, tag="zb2")
                    nc.sync.dma_start(zb[:], zb_d[ds(i * 128, 128), :])
                    nc.vector.tensor_add(zs[:], zs[:], zb[:])
                    nc.sync.dma_start(z_d[ds(i * 128, 128), :], zs[:])
                tc.For_i_unrolled(0, NB1, 1, fit_body, max_unroll=4)
    nc.compile()
    return nc


def _build_K():
    """Select (gather xn[perm]*fv -> x, xT, running max) + kNN scan."""
    B = _get_bass()
    bacc, mybir, TileContext = B["bacc"], B["mybir"], B["TileContext"]
    ds, make_identity = B["ds"], B["make_identity"]
    dt = mybir.dt
    SS = NB1 * ((128 * 1) // 16)   # select idx cols (1 slot per row)

    nc = bacc.Bacc("TRN2", target_bir_lowering=False)
    xn_d = nc.dram_tensor("xn", [XN_ROWS, HID], dt.float32,
                          kind="ExternalInput")
    sidx_d = nc.dram_tensor("sidx", [16, SS], dt.int16, kind="ExternalInput")
    fv_d = nc.dram_tensor("fv", [R1, 1], dt.float32, kind="ExternalInput")
    msk_d = nc.dram_tensor("msk", [R1, 1], dt.float32, kind="ExternalInput")
    qT_d = nc.dram_tensor("qT", [4, XT_COLS], dt.float32, kind="ExternalInput")
    cand_d = nc.dram_tensor("cand", [4, XT_COLS], dt.float32,
                            kind="ExternalInput")
    x_d = nc.dram_tensor("xo", [XN_ROWS, HID], dt.float32,
                         kind="ExternalOutput")
    xT_d = nc.dram_tensor("xT", [HID, XT_COLS], dt.float32,
                          kind="ExternalOutput")
    xsp_d = nc.dram_tensor("xsp", [128, HID], dt.float32,
                           kind="ExternalOutput")
    knn_d = nc.dram_tensor("knn", [R1, 16], dt.uint16, kind="ExternalOutput")

    with TileContext(nc) as tc:
        with (
            tc.tile_pool(name="const", bufs=1) as cpool,
            tc.tile_pool(name="ps", bufs=2, space="PSUM") as pspool,
        ):
            ident = cpool.tile([128, 128], dt.float32)
            make_identity(nc, ident[:])
            sidx_sb = cpool.tile([128, SS], dt.int16)
            for _g in range(8):
                nc.sync.dma_start(sidx_sb[_g * 16:(_g + 1) * 16, :],
                                  sidx_d[:, :])
            cand_sb = cpool.tile([4, XT_COLS], dt.float32)
            nc.sync.dma_start(cand_sb[:], cand_d[:, :])
            runmax = cpool.tile([128, HID], dt.float32)
            nc.vector.memset(runmax[:], -1e30)

            SSB = 8   # (128*1)//16
            with tc.tile_pool(name="sel", bufs=2) as wp:
                def sel_body(i):
                    g = wp.tile([128, 1, HID], dt.float32, tag="g")
                    nc.gpsimd.dma_gather(
                        out_ap=g[:], in_ap=xn_d[:, :],
                        idxs_ap=sidx_sb[:, ds(i * SSB, SSB)],
                        num_idxs=128, num_idxs_reg=128,
                        elem_size=HID, single_packet=False)
                    fv = wp.tile([128, 1], dt.float32, tag="fv")
                    nc.sync.dma_start(fv[:], fv_d[ds(i * 128, 128), :])
                    xs = wp.tile([128, HID], dt.float32, tag="xs")
                    nc.vector.tensor_scalar_mul(xs[:], g[:, 0, :], fv[:])
                    nc.sync.dma_start(x_d[ds(i * 128, 128), :], xs[:])
                    mk = wp.tile([128, 1], dt.float32, tag="mk")
                    nc.sync.dma_start(mk[:], msk_d[ds(i * 128, 128), :])
                    xm = wp.tile([128, HID], dt.float32, tag="xm2")
                    nc.vector.tensor_scalar_add(xm[:], xs[:], mk[:])
                    nc.vector.tensor_max(runmax[:], runmax[:], xm[:])
                    for c in range(4):
                        tp = pspool.tile([128, 128], dt.float32, tag="tp")
                        nc.tensor.transpose(tp[:],
                                            xs[:, c * 128:(c + 1) * 128],
                                            ident[:])
                        tt = wp.tile([128, 128], dt.float32, tag="tt")
                        nc.vector.tensor_copy(tt[:], tp[:])
                        nc.sync.dma_start(
                            xT_d[c * 128:(c + 1) * 128, ds(i * 128, 128)],
                            tt[:])
                tc.For_i_unrolled(0, NB1, 1, sel_body, max_unroll=2)
            nc.sync.dma_start(xsp_d[:, :], runmax[:])

            with tc.tile_pool(name="knn", bufs=2) as wp:
                def knn_body(i):
                    qsb = wp.tile([4, 128], dt.float32, tag="q")
                    nc.sync.dma_start(qsb[:], qT_d[:, ds(i * 128, 128)])
                    row = wp.tile([128, XT_COLS], dt.float32, tag="row")
                    for ch in range(NCH):
                        dps = pspool.tile([128, 512], dt.float32, tag="d")
                        nc.tensor.matmul(dps[:], qsb[:],
                                         cand_sb[:, ch * 512:(ch + 1) * 512],
                                         start=True, stop=True)
                        nc.scalar.activation(
                            row[:, ch * 512:(ch + 1) * 512], dps[:],
                            mybir.ActivationFunctionType.Copy)
                    v8 = wp.tile([128, 8], dt.float32, tag="v8")
                    nc.vector.max(out=v8[:], in_=row[:])
                    i16 = wp.tile([128, 16], dt.uint16, tag="i16")
                    i8 = wp.tile([128, 8], dt.uint32, tag="i8")
                    nc.vector.max_index(i8[:], v8[:], row[:])
                    nc.vector.tensor_copy(i16[:, 0:8], i8[:])
                    nc.vector.match_replace(out=row[:], in_to_replace=v8[:],
                                            in_values=row[:], imm_value=-3e30)
                    v8b = wp.tile([128, 8], dt.float32, tag="v8b")
                    nc.vector.max(out=v8b[:], in_=row[:])
                    i8b = wp.tile([128, 8], dt.uint32, tag="i8b")
                    nc.vector.max_index(i8b[:], v8b[:], row[:])
                    nc.vector.tensor_copy(i16[:, 8:16], i8b[:])
                    nc.sync.dma_start(knn_d[ds(i * 128, 128), :], i16[:])
                tc.For_i_unrolled(0, NB1, 1, knn_body, max_unroll=2)
    nc.compile()
    return nc


# ----------------------------------------------------------------------------
# build/compile management (import-time warm-up)
# ----------------------------------------------------------------------------

_RUNNERS = {}
_BUILD_LOCK = threading.Lock()
_BUILD_THREADS = []


def _get_runner(name, builder):
    with _BUILD_LOCK:
        if name in _RUNNERS:
            return _RUNNERS[name]
    r = _Launcher(builder()).warm()
    with _BUILD_LOCK:
        _RUNNERS.setdefault(name, r)
    return _RUNNERS[name]


def _warm():
    try:
        B = _get_bass()
        jnp = B["jnp"]
        ncs = {}
        ncs["L0_%d" % D0C_DEFAULT] = _build_L0(D0C_DEFAULT)
        ncs["L12"] = _build_L12()
        ncs["K"] = _build_K()
        launchers = {name: _Launcher(nc) for name, nc in ncs.items()}
        # compile the programs and the donated-zeros broadcast kernels in
        # parallel (each is a neuronx-cc subprocess)
        shapes = {}
        for l in launchers.values():
            for av in l.out_avals:
                shapes[(av.shape, str(av.dtype))] = av
        ths = [threading.Thread(target=l.warm) for l in launchers.values()]
        ths += [threading.Thread(
            target=lambda a=av: jnp.zeros(a.shape, a.dtype).block_until_ready())
            for av in shapes.values()]
        for t in ths:
            t.start()
        for t in ths:
            t.join()
        with _BUILD_LOCK:
            for name, l in launchers.items():
                _RUNNERS.setdefault(name, l)
    except Exception:  # pragma: no cover - fallback path handles
        import traceback
        traceback.print_exc()


_BUILD_THREADS.append(threading.Thread(target=_warm, daemon=True))
_BUILD_THREADS[-1].start()


# ----------------------------------------------------------------------------
# numpy fallbacks (used only if the device path fails)
# ----------------------------------------------------------------------------

def _np_reference(x, pos, src, dst, W):
    f = _f32
    n = N0
    xs = []
    for i in range(L):
        wr, br, wl = W["wr"][i], W["br"][i], W["wl"][i]
        agg = np.zeros((n, x.shape[1]), f)
        np.add.at(agg, dst, x[src])
        deg = np.bincount(dst, minlength=n).astype(f)
        mean = agg / np.maximum(deg, 1)[:, None]
        h = np.maximum(mean @ wr + br + x @ wl, 0).astype(f)
        sl = np.arange(n)
        s_ = np.concatenate([src, sl])
        d_ = np.concatenate([dst, sl])
        xj = h[s_]
        xq = np.full((n, HID), -np.inf, f)
        np.maximum.at(xq, d_, xj)
        xq = (xq @ W["lw"][i] + W["lb"][i]).astype(f)
        aw, ab = W["aw"][i], W["ab"][i]
        score = (xq[d_] @ aw[:HID] + xj @ aw[HID:] + ab).astype(f)
        score = np.where(score > 0, score, f(0.2) * score).astype(f)
        smax = np.full(n, -np.inf, f)
        np.maximum.at(smax, d_, score)
        ex = np.exp(score - smax[d_])
        ssum = np.zeros(n, f)
        np.add.at(ssum, d_, ex)
        att = (ex / ssum[d_]).astype(f)
        xn = np.zeros((n, HID), f)
        np.add.at(xn, d_, xj * att[:, None])
        a = xn @ W["l1w"][i] + W["l1b"][i]
        b = xn @ W["l2w"][i]
        agg2 = np.zeros(n, f)
        np.add.at(agg2, d_, (a[s_] - b[d_]).astype(f))
        z = (agg2 + xn @ W["l3w"][i] + W["l3b"][i]).astype(f)
        k_keep = int(math.ceil(RATIO * n))
        fit64 = 1.0 / (1.0 + np.exp(-z.astype(np.float64)))
        perm = np.argpartition(-fit64, k_keep - 1)[:k_keep]
        fv = fit64[perm].astype(f)
        x = (xn[perm] * fv[:, None]).astype(f)
        xs.append(x.max(0))
        pos = pos[perm]
        n = k_keep
        if i < L - 1:
            k = 6 + 2 * i
            sq = np.sum(pos * pos, -1)
            dist = sq[:, None] + sq[None, :] - 2 * (pos @ pos.T)
            np.fill_diagonal(dist, np.inf)
            idx = np.argpartition(dist, k, 1)[:, :k]
            srt = np.take_along_axis(dist, idx, 1).argsort(1, kind="stable")
            idx = np.take_along_axis(idx, srt, 1)
            dst = np.repeat(np.arange(n), k)
            src = idx.reshape(-1)
    return xs


# ----------------------------------------------------------------------------
# kNN host validation
# ----------------------------------------------------------------------------

def _knn_from_cand(cand16, pos, k):
    """cand16: [n, 16] device max-index results (cols sorted by -dist).
    Returns tbl [n, k] of neighbor ids; falls back per-row when needed."""
    n = pos.shape[0]
    selfid = np.arange(n, dtype=np.int64)
    c = cand16.astype(np.int64)
    not_self = c != selfid[:, None]
    # positions of first k non-self entries per row
    cum = np.cumsum(not_self, 1)
    takec = (cum <= k) & not_self
    enough = cum[:, -1] >= k
    tbl = np.zeros((n, k), np.int64)
    rows_ok = np.flatnonzero(enough)
    # fill via argsort trick: order of selected cols preserved
    sel = np.where(takec, np.arange(16)[None, :], 99)
    ordcols = np.argsort(sel, 1, kind="stable")[:, :k]
    tbl = np.take_along_axis(c, ordcols, 1)
    # validity: unique and in range
    srt = np.sort(tbl, 1)
    dup = (srt[:, 1:] == srt[:, :-1]).any(1)
    oob = (tbl < 0).any(1) | (tbl >= n).any(1)
    bad = dup | oob | ~enough
    bad_rows = np.flatnonzero(bad)
    if len(bad_rows):
        sq = np.sum(pos * pos, 1)
        for i in bad_rows:
            d = sq + sq[i] - 2.0 * (pos @ pos[i])
            d[i] = np.inf
            idx = np.argpartition(d, k)[:k]
            tbl[i] = idx[np.argsort(d[idx], kind="stable")]
    return tbl


# ----------------------------------------------------------------------------
# main kernel
# ----------------------------------------------------------------------------

_EXEC_NS = []


def kernel(x, pos, edge_index, conv0_wr, conv0_br, conv0_wl, conv_wr, conv_br,
           conv_wl, pool_lin_w, pool_lin_b, pool_att_w, pool_att_b, le1_w,
           le1_b, le2_w, le3_w, le3_b, lin1_w, lin1_b, lin2_w, lin2_b):
    t_start = time.perf_counter()
    _EXEC_NS.clear()
    x = np.asarray(x, _f32)
    pos = np.asarray(pos, _f32)
    ei = np.asarray(edge_index).astype(np.int64)

    W = {
        "wr": [np.asarray(conv0_wr, _f32)] + [np.asarray(conv_wr[i], _f32)
                                              for i in range(L - 1)],
        "br": [np.asarray(conv0_br, _f32)] + [np.asarray(conv_br[i], _f32)
                                              for i in range(L - 1)],
        "wl": [np.asarray(conv0_wl, _f32)] + [np.asarray(conv_wl[i], _f32)
                                              for i in range(L - 1)],
        "lw": [np.asarray(pool_lin_w[i], _f32) for i in range(L)],
        "lb": [np.asarray(pool_lin_b[i], _f32) for i in range(L)],
        "aw": [np.asarray(pool_att_w[i], _f32) for i in range(L)],
        "ab": [float(pool_att_b[i]) for i in range(L)],
        "l1w": [np.asarray(le1_w[i], _f32) for i in range(L)],
        "l1b": [float(le1_b[i]) for i in range(L)],
        "l2w": [np.asarray(le2_w[i], _f32) for i in range(L)],
        "l3w": [np.asarray(le3_w[i], _f32) for i in range(L)],
        "l3b": [float(le3_b[i]) for i in range(L)],
    }
    try:
        xs = _device_forward(x, pos, ei, W)
    except Exception:
        import traceback
        traceback.print_exc()
        print("kernel: device path failed; numpy fallback")
        xs = _np_reference(x, pos, ei[0], ei[1], W)

    hcat = np.concatenate(xs)[None, :].astype(_f32)
    h1 = np.maximum(hcat @ np.asarray(lin1_w, _f32) +
                    np.asarray(lin1_b, _f32), 0)
    out = (h1 @ np.asarray(lin2_w, _f32) + np.asarray(lin2_b, _f32))
    dt_ns = int((time.perf_counter() - t_start) * 1e9)
    _EXEC_NS.append(("kernel", dt_ns))
    return out.astype(_f32)


def _layer_weights(W, i):
    """Pack per-layer pool/fitness weight vectors for the L programs."""
    lw, lb = W["lw"][i], W["lb"][i]
    aw, ab = W["aw"][i], W["ab"][i]
    wq = (lw @ aw[:HID]).astype(_f32)
    qb = float(lb @ aw[:HID] + ab)
    rep = lambda v: np.ascontiguousarray(np.asarray(v, _f32))[None, :]
    return {
        "br": rep(W["br"][i]),
        "wq": rep(wq),
        "aw2": rep(aw[HID:]),
        "l1w": rep(W["l1w"][i]),
        "l2w": rep(W["l2w"][i]),
        "l3w": rep(W["l3w"][i]),
        "qb": _rep128(qb),
        "l1b": _rep128(W["l1b"][i]),
        "l3b": _rep128(W["l3b"][i]),
    }


def _device_forward(x, pos, ei, W):
    _T0 = [time.perf_counter()]
    src, dst = ei[0], ei[1]

    # ---------------- layer 0 host prep (pure numpy, overlaps warm) --------
    deg0 = np.bincount(dst, minlength=R0).astype(np.int64)
    D0C = max(int(deg0.max()), 1)
    name0 = "L0_%d" % D0C

    x0 = np.zeros((X0_ROWS, IN_CH), _f32)
    x0[:N0] = x
    SENT0 = R0
    tblC, _ = _slot_table(src, dst, R0, D0C, SENT0)
    cidx0 = _idx_to_i16_tile(_slotmajor_list(tblC))
    tblP = np.concatenate(
        [np.arange(R0, dtype=np.int64)[:, None], tblC], 1)
    tblP[N0:, 0] = SENT0   # pad rows: no self slot
    pidx0 = _idx_to_i16_tile(_slotmajor_list(tblP))
    invdeg0 = (1.0 / np.maximum(deg0, 1.0)).astype(_f32)[:, None]
    cnt0 = (deg0 + 1).astype(_f32)[:, None]
    lw0 = _layer_weights(W, 0)
    wxm = np.zeros((128, HID), _f32)
    wxm[0:IN_CH] = W["wl"][0]
    wxm[IN_CH:2 * IN_CH] = W["wr"][0]

    _EXEC_NS.append(("prep0", int((time.perf_counter() - _T0[0]) * 1e9)))
    t0 = time.perf_counter()
    for th in _BUILD_THREADS:
        th.join()
    _EXEC_NS.append(("warmjoin", int((time.perf_counter() - t0) * 1e9)))
    L0run = _RUNNERS.get(name0) or _get_runner(name0, lambda: _build_L0(D0C))
    L12run = _RUNNERS.get("L12") or _get_runner("L12", _build_L12)
    Krun = _RUNNERS.get("K") or _get_runner("K", _build_K)

    B = _get_bass()
    jax, jnp = B["jax"], B["jnp"]
    dev = jax.devices()[0]
    put = lambda a: jax.device_put(a, dev)

    t0 = time.perf_counter()
    in0 = {"x": put(x0), "cidx": put(cidx0), "pidx": put(pidx0),
           "invdeg": put(invdeg0), "cnt": put(cnt0), "wxm": put(wxm)}
    in0.update({k: put(v) for k, v in lw0.items()})
    # queue layer-1/2 weights now; transfers overlap the L0/K launches
    lw_next = {}
    for j in (1, 2):
        d = _layer_weights(W, j)
        d["wr"] = np.ascontiguousarray(
            W["wr"][j].reshape(4, 128, HID).transpose(1, 0, 2))
        d["wl"] = np.ascontiguousarray(
            W["wl"][j].reshape(4, 128, HID).transpose(1, 0, 2))
        d["invdeg"] = _rep128(1.0 / (6 + 2 * (j - 1)))
        d["cnt"] = _rep128(7 + 2 * (j - 1))
        lw_next[j] = {k: put(v) for k, v in d.items()}
    _EXEC_NS.append(("puts", int((time.perf_counter() - t0) * 1e9)))
    t0 = time.perf_counter()
    r0 = L0run(in0)
    z0 = np.asarray(r0["z"])[:N0, 0]
    _EXEC_NS.append(("L0", int((time.perf_counter() - t0) * 1e9)))

    xs_out = []
    feat_xn = r0["xn"]
    cur_pos = pos
    n_cur = N0
    for i in range(L):
        k_keep = int(math.ceil(RATIO * n_cur))
        z = z0
        # ---- host top-k ----
        perm = np.argpartition(-z, k_keep - 1)[:k_keep]
        fit = (1.0 / (1.0 + np.exp(-z[perm].astype(np.float64)))).astype(_f32)
        sel = np.zeros(R1, np.int64)
        sel[:k_keep] = perm
        fv = np.zeros((R1, 1), _f32)
        fv[:k_keep, 0] = fit
        msk = np.full((R1, 1), -1e30, _f32)
        msk[:k_keep] = 0.0
        cur_pos = cur_pos[perm]
        n_cur = k_keep
        # ---- kNN inputs ----
        if i < L - 1:
            kk = 6 + 2 * i
            sq = np.sum(cur_pos * cur_pos, 1, dtype=_f32)
            qT = np.zeros((4, XT_COLS), _f32)
            qT[0, :n_cur] = 2.0 * cur_pos[:, 0]
            qT[1, :n_cur] = 2.0 * cur_pos[:, 1]
            qT[2, :n_cur] = -1.0
            qT[3, :n_cur] = -sq
            cand = np.zeros((4, XT_COLS), _f32)
            cand[0, :n_cur] = cur_pos[:, 0]
            cand[1, :n_cur] = cur_pos[:, 1]
            cand[2, :n_cur] = sq
            cand[2, n_cur:] = 1e30
            cand[3, :] = 1.0
        else:
            kk = 0
            qT = np.zeros((4, XT_COLS), _f32)
            cand = np.zeros((4, XT_COLS), _f32)
        t0 = time.perf_counter()
        rK = Krun({"xn": feat_xn, "sidx": put(_idx_to_i16_tile(sel)),
                   "fv": put(fv), "msk": put(msk),
                   "qT": put(qT), "cand": put(cand)})
        xs_out.append(rK["xsp"])   # device partial max; reduced at the end
        _EXEC_NS.append(("K%d" % i, int((time.perf_counter() - t0) * 1e9)))
        if i == L - 1:
            break
        t0 = time.perf_counter()
        cand16 = np.asarray(rK["knn"])[:n_cur]
        _EXEC_NS.append(("knnget%d" % i, int((time.perf_counter() - t0) * 1e9)))
        t0 = time.perf_counter()
        tbl = _knn_from_cand(cand16, cur_pos, kk)
        _EXEC_NS.append(("knnval%d" % i, int((time.perf_counter() - t0) * 1e9)))

        # ---- next layer tables ----
        SENT1 = R1
        tblC1 = np.full((R1, D1C), SENT1, np.int64)
        tblC1[:n_cur, :kk] = tbl
        cidx1 = _idx_to_i16_tile(_slotmajor_list(tblC1))
        tblP1 = np.concatenate(
            [np.arange(R1, dtype=np.int64)[:, None], tblC1], 1)
        tblP1[n_cur:, 0] = SENT1
        pidx1 = _idx_to_i16_tile(_slotmajor_list(tblP1))
        inL = {"x": rK["xo"], "xT": rK["xT"],
               "cidx": put(cidx1), "pidx": put(pidx1)}
        inL.update(lw_next[i + 1])
        t0 = time.perf_counter()
        rL = L12run(inL)
        z0 = np.asarray(rL["z"])[:n_cur, 0]
        _EXEC_NS.append(("L%d" % (i + 1),
                         int((time.perf_counter() - t0) * 1e9)))
        feat_xn = rL["xn"]
    t0 = time.perf_counter()
    out = [np.asarray(p).max(0) for p in xs_out]
    _EXEC_NS.append(("xsget", int((time.perf_counter() - t0) * 1e9)))
    return out


def total_exec_ns():
    return sum(v for k, v in _EXEC_NS if k == "kernel")


def exec_breakdown():
    return list(_EXEC_NS)


# revision 15
# speedup vs baseline: 1.1186x; 1.0308x over previous
"""ASAP-GNN classifier on trn2 via Bass/Tile.

Architecture (v2): single NeuronCore, device-resident features between
launches. Three compiled programs (NEFFs), built/compiled at import time in
background threads:

  L0  : layer-0 GraphConv + ASAPool attention + LEConv fitness over the
        irregular input graph (slot-table gathers, For_i loops over 157
        row-blocks of 128 nodes).
  L12 : same pipeline for layers 1 and 2 over the fixed-degree kNN graphs
        (shared program; layer-2's 5000 nodes padded to layer-1's shape).
  K   : top-half "select" (gather xn[perm]*fv -> next x + transposed copy +
        running global max) fused with the dense kNN distance scan
        (max8/max_index, two rounds -> 16 neighbor candidates).

Host does only: slot-table construction, top-k via argpartition on the
fitness logits, kNN candidate validation, and the final 1x1536 MLP. Per
layer one launch round-trip for fitness -> perm and one for select+kNN:
6 launches total, ~KBs of traffic each after the initial ~17MB upload.
"""

import math
import threading
import time
import numpy as np

N0 = 20000
IN_CH = 64
HID = 512
OUT = 10
L = 3
RATIO = 0.5

_f32 = np.float32

# ---- geometry constants (hardcoded; program shapes) ----
NB0 = 157                   # layer-0 row blocks
R0 = NB0 * 128              # 20096
X0_ROWS = R0 + 128          # feat_x0 rows (sentinel row = R0, zeros)
D0C_DEFAULT = 17            # layer-0 max in-degree (rebuilt if actual differs)

NB1 = 79                    # layer-1/2 row blocks
R1 = NB1 * 128              # 10112
D1C = 8                     # conv slots for kNN layers (k<=8)
D1P = 9                     # pool slots (self + 8)

XN_ROWS = 20352             # unified xn/x buffer rows (>= R0 + sentinel)
XT_COLS = 10240             # x1T columns (>= R1)
NCH = XT_COLS // 512        # kNN candidate chunks (20)


# ----------------------------------------------------------------------------
# bass plumbing
# ----------------------------------------------------------------------------

_BASS = {}


def _get_bass():
    if not _BASS:
        import concourse.bass as bass
        import concourse.bacc as bacc
        import concourse.mybir as mybir
        from concourse.tile import TileContext
        from concourse.masks import make_identity
        from concourse.bass import ds
        from concourse import bass2jax
        import jax
        import jax.numpy as jnp
        bass2jax.install_neuronx_cc_hook()
        _BASS.update(bass=bass, bacc=bacc, mybir=mybir, TileContext=TileContext,
                     make_identity=make_identity, ds=ds, bass2jax=bass2jax,
                     jax=jax, jnp=jnp)
    return _BASS


class _Launcher:
    """Compiled 1-core bass program; inputs/outputs stay jax device arrays."""

    def __init__(self, nc):
        B = _get_bass()
        jax, jnp, mybir = B["jax"], B["jnp"], B["mybir"]
        bass2jax = B["bass2jax"]
        partition_name = (nc.partition_id_tensor.name
                          if nc.partition_id_tensor else None)
        in_names, in_avals, out_names, out_avals = [], [], [], []
        for alloc in nc.m.functions[0].allocations:
            if not isinstance(alloc, mybir.MemoryLocationSet):
                continue
            name = alloc.memorylocations[0].name
            if alloc.kind == "ExternalInput":
                if name != partition_name:
                    in_names.append(name)
                    in_avals.append(jax.ShapeDtypeStruct(
                        tuple(alloc.tensor_shape), mybir.dt.np(alloc.dtype)))
            elif alloc.kind == "ExternalOutput":
                out_names.append(name)
                out_avals.append(jax.core.ShapedArray(
                    tuple(alloc.tensor_shape), mybir.dt.np(alloc.dtype)))
        self.in_names = in_names
        self.in_avals = in_avals
        self.out_names = out_names
        self.out_avals = out_avals
        n_params = len(in_names)
        all_names = in_names + out_names + (
            [partition_name] if partition_name else [])
        donate = tuple(range(n_params, n_params + len(out_names)))

        def _body(*args):
            operands = list(args)
            if partition_name is not None:
                operands.append(bass2jax.partition_id_tensor())
            outs = bass2jax._bass_exec_p.bind(
                *operands, out_avals=tuple(out_avals),
                in_names=tuple(all_names), out_names=tuple(out_names),
                lowering_input_output_aliases=(),
                sim_require_finite=True, sim_require_nnan=True, nc=nc)
            return tuple(outs)

        self._jit = jax.jit(_body, donate_argnums=donate, keep_unused=True)
        self._compiled = None

    def warm(self):
        """AOT-compile the executable (no execution)."""
        B = _get_bass()
        jax = B["jax"]
        out_structs = [jax.ShapeDtypeStruct(av.shape, av.dtype)
                       for av in self.out_avals]
        self._compiled = self._jit.lower(*self.in_avals,
                                         *out_structs).compile()
        return self

    def __call__(self, in_map):
        B = _get_bass()
        jnp = B["jnp"]
        args = [in_map[nm] for nm in self.in_names]
        zeros = [jnp.zeros(av.shape, av.dtype) for av in self.out_avals]
        fn = self._compiled if self._compiled is not None else self._jit
        outs = fn(*args, *zeros)
        return dict(zip(self.out_names, outs))


# ----------------------------------------------------------------------------
# host helpers
# ----------------------------------------------------------------------------

def _idx_to_i16_tile(idx_list):
    """Compact dma_gather idx tile [16, S]: element m -> partition m%16,
    col m//16. Replicated across the 8 Q7 groups on device."""
    n = len(idx_list)
    S = (n + 15) // 16
    a = np.full((S, 16), -1, np.int16)
    a.reshape(-1)[:n] = idx_list.astype(np.int16)
    return np.ascontiguousarray(a.T)


def _slot_table(src, dst, nrows, D, sentinel):
    """[nrows, D] slot table: row i lists srcs of i's in-edges, sentinel pad."""
    deg = np.bincount(dst, minlength=nrows).astype(np.int64)
    order = np.argsort(dst, kind="stable")
    ss = src[order]
    dsrt = dst[order]
    starts = np.zeros(nrows + 1, np.int64)
    np.cumsum(deg, out=starts[1:])
    slot = np.arange(len(dsrt)) - starts[dsrt]
    tbl = np.full((nrows, D), sentinel, np.int64)
    tbl[dsrt, slot] = ss
    return tbl, deg


def _slotmajor_list(tbl):
    """[rows, D] -> block-slot-major gather list (per 128-block, slot-major)."""
    rows, D = tbl.shape
    nb = rows // 128
    return np.ascontiguousarray(
        tbl.reshape(nb, 128, D).transpose(0, 2, 1)).reshape(-1)


def _rep128(v):
    return np.full((128, 1), v, _f32)


# ----------------------------------------------------------------------------
# program builders
# ----------------------------------------------------------------------------

def _tree_sum(nc, g, n, view):
    """In-place binary-tree reduce over slot axis: view(g, lo, cnt) -> AP.
    Result lands in slot 0. Returns nothing."""
    w = n
    while w > 1:
        h = w // 2
        nc.vector.tensor_add(view(0, h), view(0, h), view(h, h))
        if w % 2:
            nc.vector.tensor_add(view(0, 1), view(0, 1), view(w - 1, 1))
        w = h


def _tree_max(nc, out_t, g, n, gview, oview):
    """Max over n slots of g into out_t (slot tile of n//2 width)."""
    h = n // 2
    nc.vector.tensor_max(oview(0, h), gview(0, h), gview(h, h))
    if n % 2:
        nc.vector.tensor_max(oview(0, 1), oview(0, 1), gview(n - 1, 1))
    w = h
    while w > 1:
        h2 = w // 2
        nc.vector.tensor_max(oview(0, h2), oview(0, h2), oview(h2, h2))
        if w % 2:
            nc.vector.tensor_max(oview(0, 1), oview(0, 1), oview(w - 1, 1))
        w = h2


def _build_L0(D0C):
    """Layer-0: conv + pool + fitness over irregular graph."""
    B = _get_bass()
    bacc, mybir, TileContext = B["bacc"], B["mybir"], B["TileContext"]
    ds, make_identity = B["ds"], B["make_identity"]
    dt = mybir.dt
    D0P = D0C + 1
    SC = NB0 * ((128 * D0C) // 16)       # conv idx tile cols
    SP = NB0 * ((128 * D0P) // 16)       # pool idx tile cols
    F = IN_CH
    HROWS = R0 + 128                     # feat_h rows, sentinel = R0

    nc = bacc.Bacc("TRN2", target_bir_lowering=False)
    x_d = nc.dram_tensor("x", [X0_ROWS, F], dt.float32, kind="ExternalInput")
    cidx_d = nc.dram_tensor("cidx", [16, SC], dt.int16, kind="ExternalInput")
    pidx_d = nc.dram_tensor("pidx", [16, SP], dt.int16, kind="ExternalInput")
    invdeg_d = nc.dram_tensor("invdeg", [R0, 1], dt.float32, kind="ExternalInput")
    cnt_d = nc.dram_tensor("cnt", [R0, 1], dt.float32, kind="ExternalInput")
    wxm_d = nc.dram_tensor("wxm", [128, HID], dt.float32, kind="ExternalInput")
    br_d = nc.dram_tensor("br", [1, HID], dt.float32, kind="ExternalInput")
    wq_d = nc.dram_tensor("wq", [1, HID], dt.float32, kind="ExternalInput")
    aw2_d = nc.dram_tensor("aw2", [1, HID], dt.float32, kind="ExternalInput")
    l1w_d = nc.dram_tensor("l1w", [1, HID], dt.float32, kind="ExternalInput")
    l2w_d = nc.dram_tensor("l2w", [1, HID], dt.float32, kind="ExternalInput")
    l3w_d = nc.dram_tensor("l3w", [1, HID], dt.float32, kind="ExternalInput")
    qb_d = nc.dram_tensor("qb", [128, 1], dt.float32, kind="ExternalInput")
    l1b_d = nc.dram_tensor("l1b", [128, 1], dt.float32, kind="ExternalInput")
    l3b_d = nc.dram_tensor("l3b", [128, 1], dt.float32, kind="ExternalInput")

    h_d = nc.dram_tensor("fh", [HROWS, 576], dt.float32, kind="Internal")
    a_d = nc.dram_tensor("fa", [HROWS, 64], dt.float32, kind="Internal")
    zb_d = nc.dram_tensor("zb", [R0, 1], dt.float32, kind="Internal")
    xn_d = nc.dram_tensor("xn", [XN_ROWS, HID], dt.float32,
                          kind="ExternalOutput")
    z_d = nc.dram_tensor("z", [R0, 1], dt.float32, kind="ExternalOutput")

    with TileContext(nc) as tc:
        with (
            tc.tile_pool(name="const", bufs=1) as cpool,
            tc.tile_pool(name="ps", bufs=2, space="PSUM") as pspool,
        ):
            ident = cpool.tile([128, 128], dt.float32)
            make_identity(nc, ident[:])
            wxm_sb = cpool.tile([128, HID], dt.float32)
            nc.sync.dma_start(wxm_sb[:], wxm_d[:, :])
            br_sb = cpool.tile([128, HID], dt.float32)
            nc.sync.dma_start(br_sb[:], br_d[0:1, :].to_broadcast([128, HID]))
            wq_sb = cpool.tile([128, HID], dt.float32)
            nc.sync.dma_start(wq_sb[:], wq_d[0:1, :].to_broadcast([128, HID]))
            aw2_sb = cpool.tile([128, HID], dt.float32)
            nc.sync.dma_start(aw2_sb[:], aw2_d[0:1, :].to_broadcast([128, HID]))
            l1w_sb = cpool.tile([128, HID], dt.float32)
            nc.sync.dma_start(l1w_sb[:], l1w_d[0:1, :].to_broadcast([128, HID]))
            l2w_sb = cpool.tile([128, HID], dt.float32)
            nc.sync.dma_start(l2w_sb[:], l2w_d[0:1, :].to_broadcast([128, HID]))
            l3w_sb = cpool.tile([128, HID], dt.float32)
            nc.sync.dma_start(l3w_sb[:], l3w_d[0:1, :].to_broadcast([128, HID]))
            qb_sb = cpool.tile([128, 1], dt.float32)
            nc.sync.dma_start(qb_sb[:], qb_d[:, :])
            l1b_sb = cpool.tile([128, 1], dt.float32)
            nc.sync.dma_start(l1b_sb[:], l1b_d[:, :])
            l3b_sb = cpool.tile([128, 1], dt.float32)
            nc.sync.dma_start(l3b_sb[:], l3b_d[:, :])
            pidx_sb = cpool.tile([128, SP], dt.int16)
            for _g in range(8):
                nc.sync.dma_start(pidx_sb[_g * 16:(_g + 1) * 16, :],
                                  pidx_d[:, :])
            # sentinel rows: feat_h[R0] = zeros except js col = -1e30;
            # feat_a[R0] = 0
            srow = cpool.tile([1, 576], dt.float32)
            nc.vector.memset(srow[:], 0.0)
            nc.vector.memset(srow[:, 512:513], -1e30)
            nc.sync.dma_start(h_d[R0:R0 + 1, :], srow[:])
            nc.sync.dma_start(a_d[R0:R0 + 1, :], srow[:, 0:64])

            # ---- phase A: conv ----
            SCB = (128 * D0C) // 16
            with tc.tile_pool(name="conv", bufs=2) as wp:
                cidx_sb = wp.tile([128, SC], dt.int16, tag="cidx", bufs=1)
                for _g in range(8):
                    nc.sync.dma_start(cidx_sb[_g * 16:(_g + 1) * 16, :],
                                      cidx_d[:, :])

                def conv_body(i):
                    g = wp.tile([128, D0C, F], dt.float32, tag="g")
                    nc.gpsimd.dma_gather(
                        out_ap=g[:], in_ap=x_d[:, :],
                        idxs_ap=cidx_sb[:, ds(i * SCB, SCB)],
                        num_idxs=128 * D0C, num_idxs_reg=128 * D0C,
                        elem_size=F, single_packet=False)
                    _tree_sum(nc, g, D0C,
                              lambda lo, cnt: g[:, lo:lo + cnt, :])
                    iv = wp.tile([128, 1], dt.float32, tag="iv")
                    nc.sync.dma_start(iv[:], invdeg_d[ds(i * 128, 128), :])
                    xm = wp.tile([128, 128], dt.float32, tag="xm")
                    nc.sync.dma_start(xm[:, 0:F], x_d[ds(i * 128, 128), :])
                    nc.vector.tensor_scalar_mul(xm[:, F:2 * F], g[:, 0, :],
                                                iv[:])
                    tp = pspool.tile([128, 128], dt.float32, tag="tp")
                    nc.tensor.transpose(tp[:], xm[:], ident[:])
                    lhsT = wp.tile([128, 128], dt.float32, tag="lhsT")
                    nc.vector.tensor_copy(lhsT[:], tp[:])
                    hps = pspool.tile([128, HID], dt.float32, tag="hps")
                    nc.tensor.matmul(hps[:], lhsT[:], wxm_sb[:],
                                     start=True, stop=True)
                    hsb = wp.tile([128, 576], dt.float32, tag="hsb")
                    nc.vector.tensor_add(
                        hsb[:, 0:HID], hps[:],
                        br_sb[:])
                    nc.vector.tensor_scalar_max(hsb[:, 0:HID], hsb[:, 0:HID],
                                                0.0)
                    tmp = wp.tile([128, HID], dt.float32, tag="tmp")
                    nc.vector.tensor_mul(tmp[:], hsb[:, 0:HID],
                                         aw2_sb[:])
                    nc.vector.tensor_reduce(hsb[:, 512:513], tmp[:],
                                            axis=mybir.AxisListType.X,
                                            op=mybir.AluOpType.add)
                    nc.sync.dma_start(h_d[ds(i * 128, 128), 0:513],
                                      hsb[:, 0:513])
                tc.For_i_unrolled(0, NB0, 1, conv_body, max_unroll=2)

            # ---- phase B: pool ----
            SPB = (128 * D0P) // 16
            with tc.tile_pool(name="pool", bufs=2) as wp:
                def pool_body(i):
                    g = wp.tile([128, D0P, 576], dt.float32, tag="g")
                    nc.gpsimd.dma_gather(
                        out_ap=g[:], in_ap=h_d[:, :],
                        idxs_ap=pidx_sb[:, ds(i * SPB, SPB)],
                        num_idxs=128 * D0P, num_idxs_reg=128 * D0P,
                        elem_size=576, single_packet=False)
                    xq = wp.tile([128, D0P // 2, HID], dt.float32, tag="xq")
                    _tree_max(nc, xq, g, D0P,
                              lambda lo, cnt: g[:, lo:lo + cnt, 0:HID],
                              lambda lo, cnt: xq[:, lo:lo + cnt, :])
                    tmp = wp.tile([128, HID], dt.float32, tag="tmp")
                    nc.vector.tensor_mul(tmp[:], xq[:, 0, :],
                                         wq_sb[:])
                    qs = wp.tile([128, 1], dt.float32, tag="qs")
                    nc.vector.tensor_reduce(qs[:], tmp[:],
                                            axis=mybir.AxisListType.X,
                                            op=mybir.AluOpType.add)
                    nc.vector.tensor_add(qs[:], qs[:], qb_sb[:])
                    # score = leaky_relu(qs + js)
                    sc = wp.tile([128, D0P], dt.float32, tag="sc")
                    jsv = g[:, :, 512:513].squeeze(2)
                    nc.vector.tensor_scalar_add(sc[:], jsv, qs[:])
                    sc2 = wp.tile([128, D0P], dt.float32, tag="sc2")
                    nc.vector.tensor_scalar_mul(sc2[:], sc[:], 0.2)
                    nc.vector.tensor_max(sc[:], sc[:], sc2[:])
                    m = wp.tile([128, 1], dt.float32, tag="m")
                    nc.vector.tensor_reduce(m[:], sc[:],
                                            axis=mybir.AxisListType.X,
                                            op=mybir.AluOpType.max)
                    nc.vector.tensor_scalar(sc[:], sc[:], m[:], None,
                                            op0=mybir.AluOpType.subtract)
                    nc.scalar.activation(sc[:], sc[:],
                                         mybir.ActivationFunctionType.Exp)
                    ssum = wp.tile([128, 1], dt.float32, tag="ssum")
                    nc.vector.tensor_reduce(ssum[:], sc[:],
                                            axis=mybir.AxisListType.X,
                                            op=mybir.AluOpType.add)
                    rec = wp.tile([128, 1], dt.float32, tag="rec")
                    nc.vector.reciprocal(rec[:], ssum[:])
                    nc.vector.tensor_scalar_mul(sc[:], sc[:], rec[:])
                    # xn = sum_s att_s * h_s  (scale slots in place, tree add)
                    gh = g[:, :, 0:HID]
                    nc.vector.tensor_mul(
                        gh, gh, sc[:].unsqueeze(2).to_broadcast(
                            [128, D0P, HID]))
                    _tree_sum(nc, g, D0P,
                              lambda lo, cnt: g[:, lo:lo + cnt, 0:HID])
                    xn = g[:, 0, 0:HID]
                    nc.sync.dma_start(xn_d[ds(i * 128, 128), :], xn)
                    # fitness scalars
                    nc.vector.tensor_mul(tmp[:], xn,
                                         l1w_sb[:])
                    av = wp.tile([128, 1], dt.float32, tag="av")
                    nc.vector.tensor_reduce(av[:], tmp[:],
                                            axis=mybir.AxisListType.X,
                                            op=mybir.AluOpType.add)
                    nc.sync.dma_start(a_d[ds(i * 128, 128), 0:1], av[:])
                    nc.vector.tensor_mul(tmp[:], xn,
                                         l2w_sb[:])
                    bv = wp.tile([128, 1], dt.float32, tag="bv")
                    nc.vector.tensor_reduce(bv[:], tmp[:],
                                            axis=mybir.AxisListType.X,
                                            op=mybir.AluOpType.add)
                    nc.vector.tensor_mul(tmp[:], xn,
                                         l3w_sb[:])
                    cv = wp.tile([128, 1], dt.float32, tag="cv")
                    nc.vector.tensor_reduce(cv[:], tmp[:],
                                            axis=mybir.AxisListType.X,
                                            op=mybir.AluOpType.add)
                    ct = wp.tile([128, 1], dt.float32, tag="ct")
                    nc.sync.dma_start(ct[:], cnt_d[ds(i * 128, 128), :])
                    # zb = c + l3b - cnt*b + cnt*l1b
                    zb = wp.tile([128, 1], dt.float32, tag="zb")
                    nc.vector.tensor_mul(zb[:], ct[:], bv[:])
                    nc.vector.tensor_sub(zb[:], cv[:], zb[:])
                    nc.vector.tensor_add(zb[:], zb[:], l3b_sb[:])
                    lb1 = wp.tile([128, 1], dt.float32, tag="lb1")
                    nc.vector.tensor_mul(lb1[:], ct[:], l1b_sb[:])
                    nc.vector.tensor_add(zb[:], zb[:], lb1[:])
                    nc.sync.dma_start(zb_d[ds(i * 128, 128), :], zb[:])
                tc.For_i_unrolled(0, NB0, 1, pool_body, max_unroll=2)

            # ---- phase C: fitness gather ----
            with tc.tile_pool(name="fit", bufs=2) as wp:
                def fit_body(i):
                    ga = wp.tile([128, D0P, 64], dt.float32, tag="ga")
                    nc.gpsimd.dma_gather(
                        out_ap=ga[:], in_ap=a_d[:, :],
                        idxs_ap=pidx_sb[:, ds(i * SPB, SPB)],
                        num_idxs=128 * D0P, num_idxs_reg=128 * D0P,
                        elem_size=64, single_packet=False)
                    zs = wp.tile([128, 1], dt.float32, tag="zs")
                    nc.vector.tensor_reduce(zs[:], ga[:, :, 0:1].squeeze(2),
                                            axis=mybir.AxisListType.X,
                                            op=mybir.AluOpType.add)
                    zb = wp.tile([128, 1], dt.float32, tag="zb2")
                    nc.sync.dma_start(zb[:], zb_d[ds(i * 128, 128), :])
                    nc.vector.tensor_add(zs[:], zs[:], zb[:])
                    nc.sync.dma_start(z_d[ds(i * 128, 128), :], zs[:])
                tc.For_i_unrolled(0, NB0, 1, fit_body, max_unroll=4)
    nc.compile()
    return nc


def _build_L12():
    """Layers 1/2: conv + pool + fitness over fixed-degree kNN graph."""
    B = _get_bass()
    bacc, mybir, TileContext = B["bacc"], B["mybir"], B["TileContext"]
    ds, make_identity = B["ds"], B["make_identity"]
    dt = mybir.dt
    F = HID
    SC = NB1 * ((128 * D1C) // 16)
    SP = NB1 * ((128 * D1P) // 16)
    HROWS = R1 + 128                    # sentinel = R1

    nc = bacc.Bacc("TRN2", target_bir_lowering=False)
    x_d = nc.dram_tensor("x", [XN_ROWS, F], dt.float32, kind="ExternalInput")
    xT_d = nc.dram_tensor("xT", [F, XT_COLS], dt.float32, kind="ExternalInput")
    cidx_d = nc.dram_tensor("cidx", [16, SC], dt.int16, kind="ExternalInput")
    pidx_d = nc.dram_tensor("pidx", [16, SP], dt.int16, kind="ExternalInput")
    invdeg_d = nc.dram_tensor("invdeg", [128, 1], dt.float32,
                              kind="ExternalInput")
    cnt_d = nc.dram_tensor("cnt", [128, 1], dt.float32, kind="ExternalInput")
    wr_d = nc.dram_tensor("wr", [128, 4, HID], dt.float32,
                          kind="ExternalInput")
    wl_d = nc.dram_tensor("wl", [128, 4, HID], dt.float32,
                          kind="ExternalInput")
    br_d = nc.dram_tensor("br", [1, HID], dt.float32, kind="ExternalInput")
    wq_d = nc.dram_tensor("wq", [1, HID], dt.float32, kind="ExternalInput")
    aw2_d = nc.dram_tensor("aw2", [1, HID], dt.float32, kind="ExternalInput")
    l1w_d = nc.dram_tensor("l1w", [1, HID], dt.float32, kind="ExternalInput")
    l2w_d = nc.dram_tensor("l2w", [1, HID], dt.float32, kind="ExternalInput")
    l3w_d = nc.dram_tensor("l3w", [1, HID], dt.float32, kind="ExternalInput")
    qb_d = nc.dram_tensor("qb", [128, 1], dt.float32, kind="ExternalInput")
    l1b_d = nc.dram_tensor("l1b", [128, 1], dt.float32, kind="ExternalInput")
    l3b_d = nc.dram_tensor("l3b", [128, 1], dt.float32, kind="ExternalInput")

    h_d = nc.dram_tensor("fh", [HROWS, 576], dt.float32, kind="Internal")
    a_d = nc.dram_tensor("fa", [HROWS, 64], dt.float32, kind="Internal")
    zb_d = nc.dram_tensor("zb", [R1, 1], dt.float32, kind="Internal")
    xn_d = nc.dram_tensor("xn", [XN_ROWS, HID], dt.float32,
                          kind="ExternalOutput")
    z_d = nc.dram_tensor("z", [R1, 1], dt.float32, kind="ExternalOutput")

    with TileContext(nc) as tc:
        with (
            tc.tile_pool(name="const", bufs=1) as cpool,
            tc.tile_pool(name="ps", bufs=2, space="PSUM") as pspool,
        ):
            ident = cpool.tile([128, 128], dt.float32)
            make_identity(nc, ident[:])
            wr_sb = cpool.tile([128, 4, HID], dt.float32)
            nc.sync.dma_start(wr_sb[:], wr_d[:, :, :])
            wl_sb = cpool.tile([128, 4, HID], dt.float32)
            nc.sync.dma_start(wl_sb[:], wl_d[:, :, :])
            br_sb = cpool.tile([128, HID], dt.float32)
            nc.sync.dma_start(br_sb[:], br_d[0:1, :].to_broadcast([128, HID]))
            wq_sb = cpool.tile([128, HID], dt.float32)
            nc.sync.dma_start(wq_sb[:], wq_d[0:1, :].to_broadcast([128, HID]))
            aw2_sb = cpool.tile([128, HID], dt.float32)
            nc.sync.dma_start(aw2_sb[:], aw2_d[0:1, :].to_broadcast([128, HID]))
            l1w_sb = cpool.tile([128, HID], dt.float32)
            nc.sync.dma_start(l1w_sb[:], l1w_d[0:1, :].to_broadcast([128, HID]))
            l2w_sb = cpool.tile([128, HID], dt.float32)
            nc.sync.dma_start(l2w_sb[:], l2w_d[0:1, :].to_broadcast([128, HID]))
            l3w_sb = cpool.tile([128, HID], dt.float32)
            nc.sync.dma_start(l3w_sb[:], l3w_d[0:1, :].to_broadcast([128, HID]))
            qb_sb = cpool.tile([128, 1], dt.float32)
            nc.sync.dma_start(qb_sb[:], qb_d[:, :])
            l1b_sb = cpool.tile([128, 1], dt.float32)
            nc.sync.dma_start(l1b_sb[:], l1b_d[:, :])
            l3b_sb = cpool.tile([128, 1], dt.float32)
            nc.sync.dma_start(l3b_sb[:], l3b_d[:, :])
            iv_sb = cpool.tile([128, 1], dt.float32)
            nc.sync.dma_start(iv_sb[:], invdeg_d[:, :])
            ct_sb = cpool.tile([128, 1], dt.float32)
            nc.sync.dma_start(ct_sb[:], cnt_d[:, :])
            cidx_sb = cpool.tile([128, SC], dt.int16)
            for _g in range(8):
                nc.sync.dma_start(cidx_sb[_g * 16:(_g + 1) * 16, :],
                                  cidx_d[:, :])
            pidx_sb = cpool.tile([128, SP], dt.int16)
            for _g in range(8):
                nc.sync.dma_start(pidx_sb[_g * 16:(_g + 1) * 16, :],
                                  pidx_d[:, :])
            srow = cpool.tile([1, 576], dt.float32)
            nc.vector.memset(srow[:], 0.0)
            nc.vector.memset(srow[:, 512:513], -1e30)
            nc.sync.dma_start(h_d[R1:R1 + 1, :], srow[:])
            nc.sync.dma_start(a_d[R1:R1 + 1, :], srow[:, 0:64])

            SCB = (128 * D1C) // 16
            SPB = (128 * D1P) // 16
            with tc.tile_pool(name="conv", bufs=2) as wp:
                def conv_body(i):
                    g = wp.tile([128, D1C, F], dt.float32, tag="g")
                    nc.gpsimd.dma_gather(
                        out_ap=g[:], in_ap=x_d[:, :],
                        idxs_ap=cidx_sb[:, ds(i * SCB, SCB)],
                        num_idxs=128 * D1C, num_idxs_reg=128 * D1C,
                        elem_size=F, single_packet=False)
                    _tree_sum(nc, g, D1C,
                              lambda lo, cnt: g[:, lo:lo + cnt, :])
                    mean = wp.tile([128, F], dt.float32, tag="mean")
                    nc.vector.tensor_scalar_mul(mean[:], g[:, 0, :], iv_sb[:])
                    hps = pspool.tile([128, HID], dt.float32, tag="hps")
                    xt = wp.tile([128, 4, 128], dt.float32, tag="xt")
                    nc.sync.dma_start(
                        xt[:], xT_d[:, ds(i * 128, 128)].rearrange(
                            "(c r) m -> r c m", c=4))
                    mt = wp.tile([128, 4, 128], dt.float32, tag="mt")
                    for c in range(4):
                        tp = pspool.tile([128, 128], dt.float32, tag="tp")
                        nc.tensor.transpose(tp[:],
                                            mean[:, c * 128:(c + 1) * 128],
                                            ident[:])
                        nc.vector.tensor_copy(mt[:, c, :], tp[:])
                    for c in range(4):
                        nc.tensor.matmul(hps[:], xt[:, c, :], wl_sb[:, c, :],
                                         start=(c == 0), stop=False)
                    for c in range(4):
                        nc.tensor.matmul(hps[:], mt[:, c, :], wr_sb[:, c, :],
                                         start=False, stop=(c == 3))
                    hsb = wp.tile([128, 576], dt.float32, tag="hsb")
                    nc.vector.tensor_add(
                        hsb[:, 0:HID], hps[:],
                        br_sb[:])
                    nc.vector.tensor_scalar_max(hsb[:, 0:HID], hsb[:, 0:HID],
                                                0.0)
                    tmp = wp.tile([128, HID], dt.float32, tag="tmp")
                    nc.vector.tensor_mul(tmp[:], hsb[:, 0:HID],
                                         aw2_sb[:])
                    nc.vector.tensor_reduce(hsb[:, 512:513], tmp[:],
                                            axis=mybir.AxisListType.X,
                                            op=mybir.AluOpType.add)
                    nc.sync.dma_start(h_d[ds(i * 128, 128), 0:513],
                                      hsb[:, 0:513])
                tc.For_i_unrolled(0, NB1, 1, conv_body, max_unroll=2)

            with tc.tile_pool(name="pool", bufs=2) as wp:
                def pool_body(i):
                    g = wp.tile([128, D1P, 576], dt.float32, tag="g")
                    nc.gpsimd.dma_gather(
                        out_ap=g[:], in_ap=h_d[:, :],
                        idxs_ap=pidx_sb[:, ds(i * SPB, SPB)],
                        num_idxs=128 * D1P, num_idxs_reg=128 * D1P,
                        elem_size=576, single_packet=False)
                    xq = wp.tile([128, D1P // 2, HID], dt.float32, tag="xq")
                    _tree_max(nc, xq, g, D1P,
                              lambda lo, cnt: g[:, lo:lo + cnt, 0:HID],
                              lambda lo, cnt: xq[:, lo:lo + cnt, :])
                    tmp = wp.tile([128, HID], dt.float32, tag="tmp")
                    nc.vector.tensor_mul(tmp[:], xq[:, 0, :],
                                         wq_sb[:])
                    qs = wp.tile([128, 1], dt.float32, tag="qs")
                    nc.vector.tensor_reduce(qs[:], tmp[:],
                                            axis=mybir.AxisListType.X,
                                            op=mybir.AluOpType.add)
                    nc.vector.tensor_add(qs[:], qs[:], qb_sb[:])
                    sc = wp.tile([128, D1P], dt.float32, tag="sc")
                    jsv = g[:, :, 512:513].squeeze(2)
                    nc.vector.tensor_scalar_add(sc[:], jsv, qs[:])
                    sc2 = wp.tile([128, D1P], dt.float32, tag="sc2")
                    nc.vector.tensor_scalar_mul(sc2[:], sc[:], 0.2)
                    nc.vector.tensor_max(sc[:], sc[:], sc2[:])
                    m = wp.tile([128, 1], dt.float32, tag="m")
                    nc.vector.tensor_reduce(m[:], sc[:],
                                            axis=mybir.AxisListType.X,
                                            op=mybir.AluOpType.max)
                    nc.vector.tensor_scalar(sc[:], sc[:], m[:], None,
                                            op0=mybir.AluOpType.subtract)
                    nc.scalar.activation(sc[:], sc[:],
                                         mybir.ActivationFunctionType.Exp)
                    ssum = wp.tile([128, 1], dt.float32, tag="ssum")
                    nc.vector.tensor_reduce(ssum[:], sc[:],
                                            axis=mybir.AxisListType.X,
                                            op=mybir.AluOpType.add)
                    rec = wp.tile([128, 1], dt.float32, tag="rec")
                    nc.vector.reciprocal(rec[:], ssum[:])
                    nc.vector.tensor_scalar_mul(sc[:], sc[:], rec[:])
                    gh = g[:, :, 0:HID]
                    nc.vector.tensor_mul(
                        gh, gh, sc[:].unsqueeze(2).to_broadcast(
                            [128, D1P, HID]))
                    _tree_sum(nc, g, D1P,
                              lambda lo, cnt: g[:, lo:lo + cnt, 0:HID])
                    xn = g[:, 0, 0:HID]
                    nc.sync.dma_start(xn_d[ds(i * 128, 128), :], xn)
                    nc.vector.tensor_mul(tmp[:], xn,
                                         l1w_sb[:])
                    av = wp.tile([128, 1], dt.float32, tag="av")
                    nc.vector.tensor_reduce(av[:], tmp[:],
                                            axis=mybir.AxisListType.X,
                                            op=mybir.AluOpType.add)
                    nc.sync.dma_start(a_d[ds(i * 128, 128), 0:1], av[:])
                    nc.vector.tensor_mul(tmp[:], xn,
                                         l2w_sb[:])
                    bv = wp.tile([128, 1], dt.float32, tag="bv")
                    nc.vector.tensor_reduce(bv[:], tmp[:],
                                            axis=mybir.AxisListType.X,
                                            op=mybir.AluOpType.add)
                    nc.vector.tensor_mul(tmp[:], xn,
                                         l3w_sb[:])
                    cv = wp.tile([128, 1], dt.float32, tag="cv")
                    nc.vector.tensor_reduce(cv[:], tmp[:],
                                            axis=mybir.AxisListType.X,
                                            op=mybir.AluOpType.add)
                    zb = wp.tile([128, 1], dt.float32, tag="zb")
                    nc.vector.tensor_mul(zb[:], ct_sb[:], bv[:])
                    nc.vector.tensor_sub(zb[:], cv[:], zb[:])
                    nc.vector.tensor_add(zb[:], zb[:], l3b_sb[:])
                    lb1 = wp.tile([128, 1], dt.float32, tag="lb1")
                    nc.vector.tensor_mul(lb1[:], ct_sb[:], l1b_sb[:])
                    nc.vector.tensor_add(zb[:], zb[:], lb1[:])
                    nc.sync.dma_start(zb_d[ds(i * 128, 128), :], zb[:])
                tc.For_i_unrolled(0, NB1, 1, pool_body, max_unroll=2)

            with tc.tile_pool(name="fit", bufs=2) as wp:
                def fit_body(i):
                    ga = wp.tile([128, D1P, 64], dt.float32, tag="ga")
                    nc.gpsimd.dma_gather(
                        out_ap=ga[:], in_ap=a_d[:, :],
                        idxs_ap=pidx_sb[:, ds(i * SPB, SPB)],
                        num_idxs=128 * D1P, num_idxs_reg=128 * D1P,
                        elem_size=64, single_packet=False)
                    zs = wp.tile([128, 1], dt.float32, tag="zs")
                    nc.vector.tensor_reduce(zs[:], ga[:, :, 0:1].squeeze(2),
                                            axis=mybir.AxisListType.X,
                                            op=mybir.AluOpType.add)
                    zb = wp.tile([128, 1], dt.float32, tag="zb2")
                    nc.sync.dma_start(zb[:], zb_d[ds(i * 128, 128), :])
                    nc.vector.tensor_add(zs[:], zs[:], zb[:])
                    nc.sync.dma_start(z_d[ds(i * 128, 128), :], zs[:])
                tc.For_i_unrolled(0, NB1, 1, fit_body, max_unroll=4)
    nc.compile()
    return nc


def _build_K():
    """Select (gather xn[perm]*fv -> x, xT, running max) + kNN scan."""
    B = _get_bass()
    bacc, mybir, TileContext = B["bacc"], B["mybir"], B["TileContext"]
    ds, make_identity = B["ds"], B["make_identity"]
    dt = mybir.dt
    SS = NB1 * ((128 * 1) // 16)   # select idx cols (1 slot per row)

    nc = bacc.Bacc("TRN2", target_bir_lowering=False)
    xn_d = nc.dram_tensor("xn", [XN_ROWS, HID], dt.float32,
                          kind="ExternalInput")
    sidx_d = nc.dram_tensor("sidx", [16, SS], dt.int16, kind="ExternalInput")
    fv_d = nc.dram_tensor("fv", [R1, 1], dt.float32, kind="ExternalInput")
    msk_d = nc.dram_tensor("msk", [R1, 1], dt.float32, kind="ExternalInput")
    qT_d = nc.dram_tensor("qT", [4, XT_COLS], dt.float32, kind="ExternalInput")
    cand_d = nc.dram_tensor("cand", [4, XT_COLS], dt.float32,
                            kind="ExternalInput")
    x_d = nc.dram_tensor("xo", [XN_ROWS, HID], dt.float32,
                         kind="ExternalOutput")
    xT_d = nc.dram_tensor("xT", [HID, XT_COLS], dt.float32,
                          kind="ExternalOutput")
    xsp_d = nc.dram_tensor("xsp", [128, HID], dt.float32,
                           kind="ExternalOutput")
    knn_d = nc.dram_tensor("knn", [R1, 16], dt.uint16, kind="ExternalOutput")

    with TileContext(nc) as tc:
        with (
            tc.tile_pool(name="const", bufs=1) as cpool,
            tc.tile_pool(name="ps", bufs=2, space="PSUM") as pspool,
        ):
            ident = cpool.tile([128, 128], dt.float32)
            make_identity(nc, ident[:])
            sidx_sb = cpool.tile([128, SS], dt.int16)
            for _g in range(8):
                nc.sync.dma_start(sidx_sb[_g * 16:(_g + 1) * 16, :],
                                  sidx_d[:, :])
            cand_sb = cpool.tile([4, XT_COLS], dt.float32)
            nc.sync.dma_start(cand_sb[:], cand_d[:, :])
            runmax = cpool.tile([128, HID], dt.float32)
            nc.vector.memset(runmax[:], -1e30)

            SSB = 8   # (128*1)//16
            with tc.tile_pool(name="sel", bufs=2) as wp:
                def sel_body(i):
                    g = wp.tile([128, 1, HID], dt.float32, tag="g")
                    nc.gpsimd.dma_gather(
                        out_ap=g[:], in_ap=xn_d[:, :],
                        idxs_ap=sidx_sb[:, ds(i * SSB, SSB)],
                        num_idxs=128, num_idxs_reg=128,
                        elem_size=HID, single_packet=False)
                    fv = wp.tile([128, 1], dt.float32, tag="fv")
                    nc.sync.dma_start(fv[:], fv_d[ds(i * 128, 128), :])
                    xs = wp.tile([128, HID], dt.float32, tag="xs")
                    nc.vector.tensor_scalar_mul(xs[:], g[:, 0, :], fv[:])
                    nc.sync.dma_start(x_d[ds(i * 128, 128), :], xs[:])
                    mk = wp.tile([128, 1], dt.float32, tag="mk")
                    nc.sync.dma_start(mk[:], msk_d[ds(i * 128, 128), :])
                    xm = wp.tile([128, HID], dt.float32, tag="xm2")
                    nc.vector.tensor_scalar_add(xm[:], xs[:], mk[:])
                    nc.vector.tensor_max(runmax[:], runmax[:], xm[:])
                    for c in range(4):
                        tp = pspool.tile([128, 128], dt.float32, tag="tp")
                        nc.tensor.transpose(tp[:],
                                            xs[:, c * 128:(c + 1) * 128],
                                            ident[:])
                        tt = wp.tile([128, 128], dt.float32, tag="tt")
                        nc.vector.tensor_copy(tt[:], tp[:])
                        nc.sync.dma_start(
                            xT_d[c * 128:(c + 1) * 128, ds(i * 128, 128)],
                            tt[:])
                tc.For_i_unrolled(0, NB1, 1, sel_body, max_unroll=2)
            nc.sync.dma_start(xsp_d[:, :], runmax[:])

            with tc.tile_pool(name="knn", bufs=2) as wp:
                def knn_body(i):
                    qsb = wp.tile([4, 128], dt.float32, tag="q")
                    nc.sync.dma_start(qsb[:], qT_d[:, ds(i * 128, 128)])
                    row = wp.tile([128, XT_COLS], dt.float32, tag="row")
                    for ch in range(NCH):
                        dps = pspool.tile([128, 512], dt.float32, tag="d")
                        nc.tensor.matmul(dps[:], qsb[:],
                                         cand_sb[:, ch * 512:(ch + 1) * 512],
                                         start=True, stop=True)
                        nc.scalar.activation(
                            row[:, ch * 512:(ch + 1) * 512], dps[:],
                            mybir.ActivationFunctionType.Copy)
                    v8 = wp.tile([128, 8], dt.float32, tag="v8")
                    nc.vector.max(out=v8[:], in_=row[:])
                    i16 = wp.tile([128, 16], dt.uint16, tag="i16")
                    i8 = wp.tile([128, 8], dt.uint32, tag="i8")
                    nc.vector.max_index(i8[:], v8[:], row[:])
                    nc.vector.tensor_copy(i16[:, 0:8], i8[:])
                    nc.vector.match_replace(out=row[:], in_to_replace=v8[:],
                                            in_values=row[:], imm_value=-3e30)
                    v8b = wp.tile([128, 8], dt.float32, tag="v8b")
                    nc.vector.max(out=v8b[:], in_=row[:])
                    i8b = wp.tile([128, 8], dt.uint32, tag="i8b")
                    nc.vector.max_index(i8b[:], v8b[:], row[:])
                    nc.vector.tensor_copy(i16[:, 8:16], i8b[:])
                    nc.sync.dma_start(knn_d[ds(i * 128, 128), :], i16[:])
                tc.For_i_unrolled(0, NB1, 1, knn_body, max_unroll=2)
    nc.compile()
    return nc


# ----------------------------------------------------------------------------
# build/compile management (import-time warm-up)
# ----------------------------------------------------------------------------

_RUNNERS = {}
_BUILD_LOCK = threading.Lock()
_BUILD_THREADS = []


def _get_runner(name, builder):
    with _BUILD_LOCK:
        if name in _RUNNERS:
            return _RUNNERS[name]
    r = _Launcher(builder()).warm()
    with _BUILD_LOCK:
        _RUNNERS.setdefault(name, r)
    return _RUNNERS[name]


def _warm():
    try:
        B = _get_bass()
        jnp = B["jnp"]
        ths = []
        zero_shapes = set()

        def _start(name, builder):
            l = _Launcher(builder())
            th = threading.Thread(target=l.warm)
            th.start()
            ths.append(th)
            for av in l.out_avals:
                key = (av.shape, str(av.dtype))
                if key not in zero_shapes:
                    zero_shapes.add(key)
                    zth = threading.Thread(
                        target=lambda a=av: jnp.zeros(
                            a.shape, a.dtype).block_until_ready())
                    zth.start()
                    ths.append(zth)
            with _BUILD_LOCK:
                _RUNNERS.setdefault(name, l)

        # build cheap programs first so their (slow) compiles overlap the
        # remaining builds; compiles run as parallel subprocesses
        _start("K", _build_K)
        _start("L12", _build_L12)
        _start("L0_%d" % D0C_DEFAULT, lambda: _build_L0(D0C_DEFAULT))
        for t in ths:
            t.join()
    except Exception:  # pragma: no cover - fallback path handles
        import traceback
        traceback.print_exc()


_BUILD_THREADS.append(threading.Thread(target=_warm, daemon=True))
_BUILD_THREADS[-1].start()


# ----------------------------------------------------------------------------
# numpy fallbacks (used only if the device path fails)
# ----------------------------------------------------------------------------

def _np_reference(x, pos, src, dst, W):
    f = _f32
    n = N0
    xs = []
    for i in range(L):
        wr, br, wl = W["wr"][i], W["br"][i], W["wl"][i]
        agg = np.zeros((n, x.shape[1]), f)
        np.add.at(agg, dst, x[src])
        deg = np.bincount(dst, minlength=n).astype(f)
        mean = agg / np.maximum(deg, 1)[:, None]
        h = np.maximum(mean @ wr + br + x @ wl, 0).astype(f)
        sl = np.arange(n)
        s_ = np.concatenate([src, sl])
        d_ = np.concatenate([dst, sl])
        xj = h[s_]
        xq = np.full((n, HID), -np.inf, f)
        np.maximum.at(xq, d_, xj)
        xq = (xq @ W["lw"][i] + W["lb"][i]).astype(f)
        aw, ab = W["aw"][i], W["ab"][i]
        score = (xq[d_] @ aw[:HID] + xj @ aw[HID:] + ab).astype(f)
        score = np.where(score > 0, score, f(0.2) * score).astype(f)
        smax = np.full(n, -np.inf, f)
        np.maximum.at(smax, d_, score)
        ex = np.exp(score - smax[d_])
        ssum = np.zeros(n, f)
        np.add.at(ssum, d_, ex)
        att = (ex / ssum[d_]).astype(f)
        xn = np.zeros((n, HID), f)
        np.add.at(xn, d_, xj * att[:, None])
        a = xn @ W["l1w"][i] + W["l1b"][i]
        b = xn @ W["l2w"][i]
        agg2 = np.zeros(n, f)
        np.add.at(agg2, d_, (a[s_] - b[d_]).astype(f))
        z = (agg2 + xn @ W["l3w"][i] + W["l3b"][i]).astype(f)
        k_keep = int(math.ceil(RATIO * n))
        fit64 = 1.0 / (1.0 + np.exp(-z.astype(np.float64)))
        perm = np.argpartition(-fit64, k_keep - 1)[:k_keep]
        fv = fit64[perm].astype(f)
        x = (xn[perm] * fv[:, None]).astype(f)
        xs.append(x.max(0))
        pos = pos[perm]
        n = k_keep
        if i < L - 1:
            k = 6 + 2 * i
            sq = np.sum(pos * pos, -1)
            dist = sq[:, None] + sq[None, :] - 2 * (pos @ pos.T)
            np.fill_diagonal(dist, np.inf)
            idx = np.argpartition(dist, k, 1)[:, :k]
            srt = np.take_along_axis(dist, idx, 1).argsort(1, kind="stable")
            idx = np.take_along_axis(idx, srt, 1)
            dst = np.repeat(np.arange(n), k)
            src = idx.reshape(-1)
    return xs


# ----------------------------------------------------------------------------
# kNN host validation
# ----------------------------------------------------------------------------

def _knn_from_cand(cand16, pos, k):
    """cand16: [n, 16] device max-index results (cols sorted by -dist).
    Returns tbl [n, k] of neighbor ids; falls back per-row when needed."""
    n = pos.shape[0]
    selfid = np.arange(n, dtype=np.int64)
    c = cand16.astype(np.int64)
    not_self = c != selfid[:, None]
    # positions of first k non-self entries per row
    cum = np.cumsum(not_self, 1)
    takec = (cum <= k) & not_self
    enough = cum[:, -1] >= k
    tbl = np.zeros((n, k), np.int64)
    rows_ok = np.flatnonzero(enough)
    # fill via argsort trick: order of selected cols preserved
    sel = np.where(takec, np.arange(16)[None, :], 99)
    ordcols = np.argsort(sel, 1, kind="stable")[:, :k]
    tbl = np.take_along_axis(c, ordcols, 1)
    # validity: unique and in range
    srt = np.sort(tbl, 1)
    dup = (srt[:, 1:] == srt[:, :-1]).any(1)
    oob = (tbl < 0).any(1) | (tbl >= n).any(1)
    bad = dup | oob | ~enough
    bad_rows = np.flatnonzero(bad)
    if len(bad_rows):
        sq = np.sum(pos * pos, 1)
        for i in bad_rows:
            d = sq + sq[i] - 2.0 * (pos @ pos[i])
            d[i] = np.inf
            idx = np.argpartition(d, k)[:k]
            tbl[i] = idx[np.argsort(d[idx], kind="stable")]
    return tbl


# ----------------------------------------------------------------------------
# main kernel
# ----------------------------------------------------------------------------

_EXEC_NS = []


def kernel(x, pos, edge_index, conv0_wr, conv0_br, conv0_wl, conv_wr, conv_br,
           conv_wl, pool_lin_w, pool_lin_b, pool_att_w, pool_att_b, le1_w,
           le1_b, le2_w, le3_w, le3_b, lin1_w, lin1_b, lin2_w, lin2_b):
    t_start = time.perf_counter()
    _EXEC_NS.clear()
    x = np.asarray(x, _f32)
    pos = np.asarray(pos, _f32)
    ei = np.asarray(edge_index).astype(np.int64)

    W = {
        "wr": [np.asarray(conv0_wr, _f32)] + [np.asarray(conv_wr[i], _f32)
                                              for i in range(L - 1)],
        "br": [np.asarray(conv0_br, _f32)] + [np.asarray(conv_br[i], _f32)
                                              for i in range(L - 1)],
        "wl": [np.asarray(conv0_wl, _f32)] + [np.asarray(conv_wl[i], _f32)
                                              for i in range(L - 1)],
        "lw": [np.asarray(pool_lin_w[i], _f32) for i in range(L)],
        "lb": [np.asarray(pool_lin_b[i], _f32) for i in range(L)],
        "aw": [np.asarray(pool_att_w[i], _f32) for i in range(L)],
        "ab": [float(pool_att_b[i]) for i in range(L)],
        "l1w": [np.asarray(le1_w[i], _f32) for i in range(L)],
        "l1b": [float(le1_b[i]) for i in range(L)],
        "l2w": [np.asarray(le2_w[i], _f32) for i in range(L)],
        "l3w": [np.asarray(le3_w[i], _f32) for i in range(L)],
        "l3b": [float(le3_b[i]) for i in range(L)],
    }
    try:
        xs = _device_forward(x, pos, ei, W)
    except Exception:
        import traceback
        traceback.print_exc()
        print("kernel: device path failed; numpy fallback")
        xs = _np_reference(x, pos, ei[0], ei[1], W)

    hcat = np.concatenate(xs)[None, :].astype(_f32)
    h1 = np.maximum(hcat @ np.asarray(lin1_w, _f32) +
                    np.asarray(lin1_b, _f32), 0)
    out = (h1 @ np.asarray(lin2_w, _f32) + np.asarray(lin2_b, _f32))
    dt_ns = int((time.perf_counter() - t_start) * 1e9)
    _EXEC_NS.append(("kernel", dt_ns))
    return out.astype(_f32)


def _layer_weights(W, i):
    """Pack per-layer pool/fitness weight vectors for the L programs."""
    lw, lb = W["lw"][i], W["lb"][i]
    aw, ab = W["aw"][i], W["ab"][i]
    wq = (lw @ aw[:HID]).astype(_f32)
    qb = float(lb @ aw[:HID] + ab)
    rep = lambda v: np.ascontiguousarray(np.asarray(v, _f32))[None, :]
    return {
        "br": rep(W["br"][i]),
        "wq": rep(wq),
        "aw2": rep(aw[HID:]),
        "l1w": rep(W["l1w"][i]),
        "l2w": rep(W["l2w"][i]),
        "l3w": rep(W["l3w"][i]),
        "qb": _rep128(qb),
        "l1b": _rep128(W["l1b"][i]),
        "l3b": _rep128(W["l3b"][i]),
    }


def _device_forward(x, pos, ei, W):
    _T0 = [time.perf_counter()]
    src, dst = ei[0], ei[1]

    # ---------------- layer 0 host prep (pure numpy, overlaps warm) --------
    deg0 = np.bincount(dst, minlength=R0).astype(np.int64)
    D0C = max(int(deg0.max()), 1)
    name0 = "L0_%d" % D0C

    x0 = np.zeros((X0_ROWS, IN_CH), _f32)
    x0[:N0] = x
    SENT0 = R0
    tblC, _ = _slot_table(src, dst, R0, D0C, SENT0)
    cidx0 = _idx_to_i16_tile(_slotmajor_list(tblC))
    tblP = np.concatenate(
        [np.arange(R0, dtype=np.int64)[:, None], tblC], 1)
    tblP[N0:, 0] = SENT0   # pad rows: no self slot
    pidx0 = _idx_to_i16_tile(_slotmajor_list(tblP))
    invdeg0 = (1.0 / np.maximum(deg0, 1.0)).astype(_f32)[:, None]
    cnt0 = (deg0 + 1).astype(_f32)[:, None]
    lw0 = _layer_weights(W, 0)
    wxm = np.zeros((128, HID), _f32)
    wxm[0:IN_CH] = W["wl"][0]
    wxm[IN_CH:2 * IN_CH] = W["wr"][0]

    _EXEC_NS.append(("prep0", int((time.perf_counter() - _T0[0]) * 1e9)))
    t0 = time.perf_counter()
    for th in _BUILD_THREADS:
        th.join()
    _EXEC_NS.append(("warmjoin", int((time.perf_counter() - t0) * 1e9)))
    L0run = _RUNNERS.get(name0) or _get_runner(name0, lambda: _build_L0(D0C))
    L12run = _RUNNERS.get("L12") or _get_runner("L12", _build_L12)
    Krun = _RUNNERS.get("K") or _get_runner("K", _build_K)

    B = _get_bass()
    jax, jnp = B["jax"], B["jnp"]
    dev = jax.devices()[0]
    put = lambda a: jax.device_put(a, dev)

    t0 = time.perf_counter()
    in0 = {"x": put(x0), "cidx": put(cidx0), "pidx": put(pidx0),
           "invdeg": put(invdeg0), "cnt": put(cnt0), "wxm": put(wxm)}
    in0.update({k: put(v) for k, v in lw0.items()})
    # queue layer-1/2 weights now; transfers overlap the L0/K launches
    lw_next = {}
    for j in (1, 2):
        d = _layer_weights(W, j)
        d["wr"] = np.ascontiguousarray(
            W["wr"][j].reshape(4, 128, HID).transpose(1, 0, 2))
        d["wl"] = np.ascontiguousarray(
            W["wl"][j].reshape(4, 128, HID).transpose(1, 0, 2))
        d["invdeg"] = _rep128(1.0 / (6 + 2 * (j - 1)))
        d["cnt"] = _rep128(7 + 2 * (j - 1))
        lw_next[j] = {k: put(v) for k, v in d.items()}
    _EXEC_NS.append(("puts", int((time.perf_counter() - t0) * 1e9)))
    t0 = time.perf_counter()
    r0 = L0run(in0)
    z0 = np.asarray(r0["z"])[:N0, 0]
    _EXEC_NS.append(("L0", int((time.perf_counter() - t0) * 1e9)))

    xs_out = []
    feat_xn = r0["xn"]
    cur_pos = pos
    n_cur = N0
    for i in range(L):
        k_keep = int(math.ceil(RATIO * n_cur))
        z = z0
        # ---- host top-k ----
        perm = np.argpartition(-z, k_keep - 1)[:k_keep]
        fit = (1.0 / (1.0 + np.exp(-z[perm].astype(np.float64)))).astype(_f32)
        sel = np.zeros(R1, np.int64)
        sel[:k_keep] = perm
        fv = np.zeros((R1, 1), _f32)
        fv[:k_keep, 0] = fit
        msk = np.full((R1, 1), -1e30, _f32)
        msk[:k_keep] = 0.0
        cur_pos = cur_pos[perm]
        n_cur = k_keep
        # ---- kNN inputs ----
        if i < L - 1:
            kk = 6 + 2 * i
            sq = np.sum(cur_pos * cur_pos, 1, dtype=_f32)
            qT = np.zeros((4, XT_COLS), _f32)
            qT[0, :n_cur] = 2.0 * cur_pos[:, 0]
            qT[1, :n_cur] = 2.0 * cur_pos[:, 1]
            qT[2, :n_cur] = -1.0
            qT[3, :n_cur] = -sq
            cand = np.zeros((4, XT_COLS), _f32)
            cand[0, :n_cur] = cur_pos[:, 0]
            cand[1, :n_cur] = cur_pos[:, 1]
            cand[2, :n_cur] = sq
            cand[2, n_cur:] = 1e30
            cand[3, :] = 1.0
        else:
            kk = 0
            qT = np.zeros((4, XT_COLS), _f32)
            cand = np.zeros((4, XT_COLS), _f32)
        t0 = time.perf_counter()
        rK = Krun({"xn": feat_xn, "sidx": put(_idx_to_i16_tile(sel)),
                   "fv": put(fv), "msk": put(msk),
                   "qT": put(qT), "cand": put(cand)})
        xs_out.append(rK["xsp"])   # device partial max; reduced at the end
        _EXEC_NS.append(("K%d" % i, int((time.perf_counter() - t0) * 1e9)))
        if i == L - 1:
            break
        t0 = time.perf_counter()
        cand16 = np.asarray(rK["knn"])[:n_cur]
        _EXEC_NS.append(("knnget%d" % i, int((time.perf_counter() - t0) * 1e9)))
        t0 = time.perf_counter()
        tbl = _knn_from_cand(cand16, cur_pos, kk)
        _EXEC_NS.append(("knnval%d" % i, int((time.perf_counter() - t0) * 1e9)))

        # ---- next layer tables ----
        SENT1 = R1
        tblC1 = np.full((R1, D1C), SENT1, np.int64)
        tblC1[:n_cur, :kk] = tbl
        cidx1 = _idx_to_i16_tile(_slotmajor_list(tblC1))
        tblP1 = np.concatenate(
            [np.arange(R1, dtype=np.int64)[:, None], tblC1], 1)
        tblP1[n_cur:, 0] = SENT1
        pidx1 = _idx_to_i16_tile(_slotmajor_list(tblP1))
        inL = {"x": rK["xo"], "xT": rK["xT"],
               "cidx": put(cidx1), "pidx": put(pidx1)}
        inL.update(lw_next[i + 1])
        t0 = time.perf_counter()
        rL = L12run(inL)
        z0 = np.asarray(rL["z"])[:n_cur, 0]
        _EXEC_NS.append(("L%d" % (i + 1),
                         int((time.perf_counter() - t0) * 1e9)))
        feat_xn = rL["xn"]
    t0 = time.perf_counter()
    xs_host = jax.device_get(xs_out)
    out = [np.asarray(p).max(0) for p in xs_host]
    _EXEC_NS.append(("xsget", int((time.perf_counter() - t0) * 1e9)))
    return out


def total_exec_ns():
    return sum(v for k, v in _EXEC_NS if k == "kernel")


def exec_breakdown():
    return list(_EXEC_NS)
